# revision 1
# baseline (speedup 1.0000x reference)
"""Trainium2 Bass kernel for a dense transformer encoder layer.

Model (faithful to the oracle):
  q,k,v = x@wq+bq, x@wk+bk, x@wv+bv          (12 heads, dk=64, DIM=768)
  scores = q@k^T / sqrt(768)  (note: sqrt(dim_model), not sqrt(dk))
  scores[mask==0] = 1e-11  (NOT -inf; masked keys still contribute ~1/Z)
  attn = softmax(scores); z = attn@v; o = z@wo+bo
  l1 = x + LN(o);  ffn = relu(l1@w1+b1)@w2+b2;  out = l1 + LN(ffn)

Sharding: 4096 tokens (B=2,S=2048) split 8 ways -> 512 tokens/core.
Cores 0-3 own batch 0, cores 4-7 batch 1. K/V are computed for the
core's whole batch (redundantly within each 4-core group) so attention
needs no collectives.

Softmax trick: scores are built k-major (scoresT [kpos, q]) so the
mask (per-k) is a per-partition scalar; exp(mask_p/sqrt(768) * s) on
the scalar engine applies scale+mask+exp in a single pass (masked rows
give exp(0)=1.0 == fp32(exp(1e-11))). The denominator comes from a
ones column appended to V (attn@v with M=65); normalization happens
after attn@v via a rank-1 matmul broadcast of 1/sum.
"""

import math
import os
import sys

import numpy as np

for _p in ("/opt/trn_rl_repo", os.path.expanduser("~/.axon_site/_ro/trn_rl_repo")):
    if os.path.isdir(_p) and _p not in sys.path:
        sys.path.insert(0, _p)

import ml_dtypes  # noqa: E402

BF16 = ml_dtypes.bfloat16

DIM = 768
HEADS = 12
DK = 64
HID = 4 * DIM  # 3072
B, S = 2, 2048
N_CORES = 8
BLK = 512            # tokens per core
NBLK = S // BLK      # 4 blocks per batch
EPS = 1e-5
ISCALE = 1.0 / math.sqrt(DIM)

_CACHE: dict = {}
MAX_PHASE = int(os.environ.get("BASS_KERNEL_PHASES", "5"))
USE_AG = os.environ.get("BASS_KERNEL_AG", "1") == "1"


def _build_program():
    import concourse.bass as bass
    import concourse.mybir as mybir
    import concourse.tile as tile
    from concourse import bacc
    from concourse.masks import make_identity

    f32 = mybir.dt.float32
    bf16 = mybir.dt.bfloat16
    AF = mybir.ActivationFunctionType
    ALU = mybir.AluOpType
    AX = mybir.AxisListType

    nc = bacc.Bacc()

    # ---- per-core DRAM I/O ----
    if not USE_AG:
        d_xT = nc.dram_tensor("xT", [DIM, S], bf16, kind="ExternalInput")
    d_xTb = nc.dram_tensor("xTb", [DIM, BLK], bf16, kind="ExternalInput")
    d_xb = nc.dram_tensor("xb", [BLK, DIM], f32, kind="ExternalInput")
    d_msc = nc.dram_tensor("msc", [S], f32, kind="ExternalInput")
    d_wq = nc.dram_tensor("wq", [DIM, DIM], bf16, kind="ExternalInput")
    d_wk = nc.dram_tensor("wk", [DIM, DIM], bf16, kind="ExternalInput")
    d_wv = nc.dram_tensor("wv", [DIM, DIM], bf16, kind="ExternalInput")
    d_wo = nc.dram_tensor("wo", [DIM, DIM], bf16, kind="ExternalInput")
    d_w1 = nc.dram_tensor("w1", [DIM, HID], bf16, kind="ExternalInput")
    d_w2 = nc.dram_tensor("w2", [HID, DIM], bf16, kind="ExternalInput")
    d_bq = nc.dram_tensor("bq", [DIM], f32, kind="ExternalInput")
    d_bk = nc.dram_tensor("bk", [DIM], f32, kind="ExternalInput")
    d_bv = nc.dram_tensor("bv", [DIM], f32, kind="ExternalInput")
    d_bo = nc.dram_tensor("bo", [DIM], f32, kind="ExternalInput")
    d_b1 = nc.dram_tensor("b1", [HID], f32, kind="ExternalInput")
    d_b2 = nc.dram_tensor("b2", [DIM], f32, kind="ExternalInput")
    d_g1 = nc.dram_tensor("g1", [DIM], f32, kind="ExternalInput")
    d_bb1 = nc.dram_tensor("bb1", [DIM], f32, kind="ExternalInput")
    d_g2 = nc.dram_tensor("g2", [DIM], f32, kind="ExternalInput")
    d_bb2 = nc.dram_tensor("bb2", [DIM], f32, kind="ExternalInput")
    d_out = nc.dram_tensor("out", [BLK, DIM], f32, kind="ExternalOutput")
    if USE_AG:
        d_kb = nc.dram_tensor("k_bounce", [DIM, BLK], bf16)
        d_ks = nc.dram_tensor("k_shared", [NBLK * DIM, BLK], bf16)
        d_vb = nc.dram_tensor("v_bounce", [BLK, HEADS * (DK + 1)], bf16)
        d_vs = nc.dram_tensor("v_shared", [S, HEADS * (DK + 1)], bf16)
        RG = [[0, 1, 2, 3], [4, 5, 6, 7]]

    FT = DIM // 128   # 6 feature tiles
    TT = BLK // 128   # 4 token tiles per core block
    ST = S // 128     # 16 token tiles per batch
    HT = HID // 128   # 24 hidden tiles

    def bcast_ap(handle, n=128):
        ap = handle[:]
        return bass.AP(tensor=ap.tensor, offset=ap.offset, ap=[[0, n]] + list(ap.ap))

    with tile.TileContext(nc) as tc:
        with (
            tc.tile_pool(name="const", bufs=1) as const,
            tc.tile_pool(name="bigres", bufs=1) as big,
        ):
            # ---------- constants ----------
            sb_msc = const.tile([128, ST], f32)
            nc.sync.dma_start(out=sb_msc, in_=d_msc[:].rearrange("(t p) -> p t", p=128))
            sb_bq = const.tile([128, FT], f32)
            nc.sync.dma_start(out=sb_bq, in_=d_bq[:].rearrange("(t p) -> p t", p=128))
            sb_bk = const.tile([128, FT], f32)
            nc.sync.dma_start(out=sb_bk, in_=d_bk[:].rearrange("(t p) -> p t", p=128))
            sb_b1 = const.tile([128, HT], f32)
            nc.sync.dma_start(out=sb_b1, in_=d_b1[:].rearrange("(t p) -> p t", p=128))
            bv_bc = const.tile([128, DIM], f32)
            nc.gpsimd.dma_start(out=bv_bc, in_=bcast_ap(d_bv))
            bo_bc = const.tile([128, DIM], f32)
            nc.gpsimd.dma_start(out=bo_bc, in_=bcast_ap(d_bo))
            b2_bc = const.tile([128, DIM], f32)
            nc.gpsimd.dma_start(out=b2_bc, in_=bcast_ap(d_b2))
            g1_bc = const.tile([128, DIM], f32)
            nc.gpsimd.dma_start(out=g1_bc, in_=bcast_ap(d_g1))
            bb1_bc = const.tile([128, DIM], f32)
            nc.gpsimd.dma_start(out=bb1_bc, in_=bcast_ap(d_bb1))
            g2_bc = const.tile([128, DIM], f32)
            nc.gpsimd.dma_start(out=g2_bc, in_=bcast_ap(d_g2))
            bb2_bc = const.tile([128, DIM], f32)
            nc.gpsimd.dma_start(out=bb2_bc, in_=bcast_ap(d_bb2))
            ident = const.tile([128, 128], f32)
            make_identity(nc, ident[:])
            ones64 = const.tile([1, 64], f32)
            nc.vector.memset(ones64, 1.0)
            eps_t = const.tile([128, 1], f32)
            nc.vector.memset(eps_t, EPS)

            # ---------- persistent activations ----------
            sb_xblk = big.tile([128, TT, DIM], f32)  # residual x
            sb_l1 = big.tile([128, TT, DIM], f32)

            nc.sync.dma_start(
                out=sb_xblk, in_=d_xb[:].rearrange("(t p) d -> p t d", p=128)
            )

            # attention-scoped residents (freed before the FFN phases)
            attn_res_cm = tc.tile_pool(name="attn_res", bufs=1)
            attn_res = attn_res_cm.__enter__()
            sb_K = attn_res.tile([128, FT, NBLK, BLK], bf16)  # K^T, feat-major
            sb_Q = attn_res.tile([128, FT, BLK], bf16)  # Q^T, feat-major
            sb_V = attn_res.tile([128, ST, HEADS, DK + 1], bf16)  # V + ones col
            sb_zT = attn_res.tile([128, FT, BLK], bf16)  # z^T normalized

            # ============ Phase 1: QKV projections ============
            with (
                tc.tile_pool(name="xw", bufs=1) as xw,
                tc.tile_pool(name="ps1", bufs=4, space="PSUM") as ps1,
                tc.tile_pool(name="ps1v", bufs=4, space="PSUM") as ps1v,
            ):
                if not USE_AG:
                    sb_xT = xw.tile([128, FT, S], bf16)
                    nc.sync.dma_start(
                        out=sb_xT, in_=d_xT[:].rearrange("(t p) n -> p t n", p=128)
                    )
                sb_xTb = xw.tile([128, FT, BLK], bf16)
                nc.sync.dma_start(
                    out=sb_xTb, in_=d_xTb[:].rearrange("(t p) n -> p t n", p=128)
                )
                w_q = xw.tile([128, FT, DIM], bf16)
                nc.sync.dma_start(
                    out=w_q, in_=d_wq[:].rearrange("(t p) o -> p t o", p=128)
                )
                w_k = xw.tile([128, FT, DIM], bf16)
                nc.sync.dma_start(
                    out=w_k, in_=d_wk[:].rearrange("(t p) o -> p t o", p=128)
                )
                w_v = xw.tile([128, FT, DIM], bf16)
                nc.sync.dma_start(
                    out=w_v, in_=d_wv[:].rearrange("(t p) o -> p t o", p=128)
                )

                if USE_AG:
                    # K^T feat-major for the own block only -> bounce -> AG
                    kstage = xw.tile([128, FT, BLK], bf16, tag="kstage")
                    for ft in range(FT):
                        ps = ps1.tile([128, 512], f32, tag="p")
                        for kt in range(FT):
                            nc.tensor.matmul(
                                ps,
                                w_k[:, kt, ft * 128 : (ft + 1) * 128],
                                sb_xTb[:, kt, :],
                                start=(kt == 0),
                                stop=(kt == FT - 1),
                            )
                        nc.vector.tensor_scalar_add(
                            kstage[:, ft, :], ps, sb_bk[:, ft : ft + 1]
                        )
                    nc.sync.dma_start(
                        out=d_kb[:].rearrange("(t p) n -> p t n", p=128), in_=kstage
                    )
                else:
                    # K^T feat-major over the whole batch (replicated)
                    for ft in range(FT):
                        for nt in range(S // 512):
                            ps = ps1.tile([128, 512], f32, tag="p")
                            for kt in range(FT):
                                nc.tensor.matmul(
                                    ps,
                                    w_k[:, kt, ft * 128 : (ft + 1) * 128],
                                    sb_xT[:, kt, nt * 512 : (nt + 1) * 512],
                                    start=(kt == 0),
                                    stop=(kt == FT - 1),
                                )
                            nc.vector.tensor_scalar_add(
                                sb_K[:, ft, nt, :], ps, sb_bk[:, ft : ft + 1]
                            )
                # Q^T feat-major for the core's block
                for ft in range(FT):
                    ps = ps1.tile([128, 512], f32, tag="p")
                    for kt in range(FT):
                        nc.tensor.matmul(
                            ps,
                            w_q[:, kt, ft * 128 : (ft + 1) * 128],
                            sb_xTb[:, kt, :],
                            start=(kt == 0),
                            stop=(kt == FT - 1),
                        )
                    nc.vector.tensor_scalar_add(
                        sb_Q[:, ft, :], ps, sb_bq[:, ft : ft + 1]
                    )
                if USE_AG:
                    # V tok-major for the own block -> bounce -> AG
                    vstage = xw.tile([128, TT, HEADS, DK + 1], bf16, tag="vstage")
                    nc.vector.memset(vstage[:, :, :, DK : DK + 1], 1.0)
                    for tt in range(TT):
                        for nh in range(2):
                            ps = ps1v.tile([128, 384], f32, tag="vp")
                            for kt in range(FT):
                                nc.tensor.matmul(
                                    ps,
                                    sb_xTb[:, kt, tt * 128 : (tt + 1) * 128],
                                    w_v[:, kt, nh * 384 : (nh + 1) * 384],
                                    start=(kt == 0),
                                    stop=(kt == FT - 1),
                                )
                            nc.vector.scalar_tensor_tensor(
                                out=vstage[:, tt, nh * 6 : (nh + 1) * 6, 0:DK],
                                in0=ps[:].rearrange("p (h d) -> p h d", d=DK),
                                scalar=1.0,
                                in1=bv_bc[:, nh * 384 : (nh + 1) * 384].rearrange(
                                    "p (h d) -> p h d", d=DK
                                ),
                                op0=ALU.mult,
                                op1=ALU.add,
                            )
                    nc.sync.dma_start(
                        out=d_vb[:].rearrange("(t p) (h d) -> p t h d", p=128, d=DK + 1),
                        in_=vstage,
                    )
                    # AllGather K and V across the 4-core batch group
                    nc.gpsimd.collective_compute(
                        "AllGather", ALU.bypass, replica_groups=RG,
                        ins=[d_kb[:]], outs=[d_ks[:]],
                    )
                    nc.gpsimd.collective_compute(
                        "AllGather", ALU.bypass, replica_groups=RG,
                        ins=[d_vb[:]], outs=[d_vs[:]],
                    )
                    for b in range(NBLK):
                        nc.sync.dma_start(
                            out=sb_K[:, :, b, :],
                            in_=d_ks[b * DIM : (b + 1) * DIM, :].rearrange(
                                "(t p) n -> p t n", p=128
                            ),
                        )
                    nc.sync.dma_start(
                        out=sb_V,
                        in_=d_vs[:].rearrange(
                            "(t p) (h d) -> p t h d", p=128, d=DK + 1
                        ),
                    )
                else:
                    # V tok-major over the whole batch, laid out [tok, head, dk+1]
                    nc.vector.memset(sb_V[:, :, :, DK : DK + 1], 1.0)
                    for nh in range(2):
                        for tt in range(ST):
                            ps = ps1v.tile([128, 384], f32, tag="vp")
                            for kt in range(FT):
                                nc.tensor.matmul(
                                    ps,
                                    sb_xT[:, kt, tt * 128 : (tt + 1) * 128],
                                    w_v[:, kt, nh * 384 : (nh + 1) * 384],
                                    start=(kt == 0),
                                    stop=(kt == FT - 1),
                                )
                            nc.vector.scalar_tensor_tensor(
                                out=sb_V[:, tt, nh * 6 : (nh + 1) * 6, 0:DK],
                                in0=ps[:].rearrange("p (h d) -> p h d", d=DK),
                                scalar=1.0,
                                in1=bv_bc[:, nh * 384 : (nh + 1) * 384].rearrange(
                                    "p (h d) -> p h d", d=DK
                                ),
                                op0=ALU.mult,
                                op1=ALU.add,
                            )

            if MAX_PHASE >= 2:
                # ============ Phase 2: attention ============
                with (
                    tc.tile_pool(name="expp", bufs=64) as expp,
                    tc.tile_pool(name="attsm", bufs=2) as attsm,
                    tc.tile_pool(name="ps_sc", bufs=4, space="PSUM") as ps_sc,
                    tc.tile_pool(name="ps_z", bufs=2, space="PSUM") as ps_z,
                    tc.tile_pool(name="ps_rb", bufs=1, space="PSUM") as ps_rb,
                ):
                    for hp in range(HEADS // 2):
                        ht = hp
                        # interleave the two heads of a pair kt-by-kt: their
                        # K=64 matmuls sit in disjoint PE row groups (0-63 /
                        # 64-127) so the hardware overlaps adjacent pairs.
                        ets = ([], [])
                        for kt2 in range(ST):
                            for half in (0, 1):
                                ho = half * 64
                                ps = ps_sc.tile([128, BLK], f32, tag="sc")
                                nc.tensor.matmul(
                                    ps,
                                    sb_K[ho : ho + 64, ht, kt2 // 4, (kt2 % 4) * 128 : (kt2 % 4) * 128 + 128],
                                    sb_Q[ho : ho + 64, ht, :],
                                    start=True,
                                    stop=True,
                                )
                                et = expp.tile([128, BLK], bf16, tag="exp")
                                nc.scalar.activation(
                                    et, ps, AF.Exp, scale=sb_msc[:, kt2 : kt2 + 1]
                                )
                                ets[half].append(et)
                        for half in (0, 1):
                            h = 2 * hp + half
                            ho = half * 64
                            zp = ps_z.tile([DK + 1, BLK], f32, tag="z")
                            for kt2 in range(ST):
                                nc.tensor.matmul(
                                    zp,
                                    sb_V[:, kt2, h, :],
                                    ets[half][kt2],
                                    start=(kt2 == 0),
                                    stop=(kt2 == ST - 1),
                                )
                            rsum = attsm.tile([1, BLK], f32, tag="rsum")
                            nc.vector.reciprocal(rsum, zp[DK : DK + 1, :])
                            rbp = ps_rb.tile([64, BLK], f32, tag="rb")
                            nc.tensor.matmul(
                                rbp, ones64[:], rsum, start=True, stop=True
                            )
                            rb = attsm.tile([64, BLK], f32, tag="rbs")
                            nc.vector.tensor_copy(rb, rbp)
                            nc.vector.tensor_mul(
                                sb_zT[ho : ho + 64, ht, :], zp[0:DK, :], rb
                            )

            if MAX_PHASE >= 3:
                # ============ Phase 3: O proj + LN1 (+residual) ============
                def layer_norm_to(out_ap, x_ap, g_bc_t, resid_ap, pool):
                    s = pool.tile([128, 1], f32, tag="ln_s")
                    nc.vector.tensor_reduce(s, x_ap, axis=AX.X, op=ALU.add)
                    mean = pool.tile([128, 1], f32, tag="ln_m")
                    nc.scalar.mul(mean, s, 1.0 / DIM)
                    xc = pool.tile([128, DIM], f32, tag="ln_xc")
                    nc.vector.tensor_scalar(xc, x_ap, mean, None, op0=ALU.subtract)
                    junk = pool.tile([128, DIM], f32, tag="ln_j")
                    var = pool.tile([128, 1], f32, tag="ln_v")
                    # (tensor_tensor_reduce crashes the device on this runtime;
                    # scalar_tensor_tensor with accum_out works)
                    nc.vector.scalar_tensor_tensor(
                        out=junk, in0=xc, scalar=1.0, in1=xc,
                        op0=ALU.mult, op1=ALU.mult, accum_out=var,
                    )
                    nc.vector.tensor_scalar_mul(var, var, 1.0 / DIM)
                    sd = pool.tile([128, 1], f32, tag="ln_sd")
                    nc.scalar.activation(sd, var, AF.Sqrt, bias=eps_t[:])
                    rstd = pool.tile([128, 1], f32, tag="ln_r")
                    nc.vector.reciprocal(rstd, sd)
                    t = pool.tile([128, DIM], f32, tag="ln_t")
                    nc.vector.tensor_scalar(t, xc, rstd, None, op0=ALU.mult)
                    tg = pool.tile([128, DIM], f32, tag="ln_tg")
                    nc.vector.tensor_mul(tg, t, g_bc_t)
                    nc.vector.tensor_add(out_ap, tg, resid_ap)

                with (
                    tc.tile_pool(name="wo_p", bufs=1) as wo_p,
                    tc.tile_pool(name="ln1p", bufs=2) as ln1p,
                    tc.tile_pool(name="ps_o", bufs=4, space="PSUM") as ps_o,
                ):
                    w_o = wo_p.tile([128, FT, DIM], bf16)
                    nc.sync.dma_start(
                        out=w_o, in_=d_wo[:].rearrange("(t p) o -> p t o", p=128)
                    )
                    for tt in range(TT):
                        l1pre = ln1p.tile([128, DIM], f32, tag="l1pre")
                        for nh in range(2):
                            ps = ps_o.tile([128, 384], f32, tag="op")
                            for kt in range(FT):
                                nc.tensor.matmul(
                                    ps,
                                    sb_zT[:, kt, tt * 128 : (tt + 1) * 128],
                                    w_o[:, kt, nh * 384 : (nh + 1) * 384],
                                    start=(kt == 0),
                                    stop=(kt == FT - 1),
                                )
                            nc.vector.scalar_tensor_tensor(
                                out=l1pre[:, nh * 384 : (nh + 1) * 384],
                                in0=ps,
                                scalar=1.0,
                                in1=bo_bc[:, nh * 384 : (nh + 1) * 384],
                                op0=ALU.mult,
                                op1=ALU.add,
                            )
                        xb1 = ln1p.tile([128, DIM], f32, tag="xb1")
                        nc.vector.tensor_add(xb1, sb_xblk[:, tt, :], bb1_bc)
                        layer_norm_to(sb_l1[:, tt, :], l1pre[:], g1_bc, xb1, ln1p)

            attn_res_cm.__exit__(None, None, None)
            sb_hT = big.tile([128, HT, BLK], bf16)  # relu(ffn1)^T, hid-major

            if MAX_PHASE >= 4:
                # ============ Phase 4: transpose l1, FFN1 ============
                with (
                    tc.tile_pool(name="w1_p", bufs=1) as w1_p,
                    tc.tile_pool(name="l1t_p", bufs=1) as l1t_p,
                    tc.tile_pool(name="ps_t", bufs=2, space="PSUM") as ps_t,
                    tc.tile_pool(name="ps_f1", bufs=4, space="PSUM") as ps_f1,
                ):
                    w1_t = []
                    for kt in range(FT):
                        wt = w1_p.tile([128, HID], bf16, tag=f"w1_{kt}")
                        nc.sync.dma_start(
                            out=wt, in_=d_w1[kt * 128 : (kt + 1) * 128, :]
                        )
                        w1_t.append(wt)
                    sb_l1T = l1t_p.tile([128, FT, BLK], bf16)
                    for ft in range(FT):
                        for tt in range(TT):
                            pst = ps_t.tile([128, 128], f32, tag="tp")
                            nc.tensor.transpose(
                                pst, sb_l1[:, tt, ft * 128 : (ft + 1) * 128], ident[:]
                            )
                            nc.scalar.copy(
                                sb_l1T[:, ft, tt * 128 : (tt + 1) * 128], pst
                            )
                    for ht2 in range(HT):
                        ps = ps_f1.tile([128, BLK], f32, tag="f1")
                        for kt in range(FT):
                            nc.tensor.matmul(
                                ps,
                                w1_t[kt][:, ht2 * 128 : (ht2 + 1) * 128],
                                sb_l1T[:, kt, :],
                                start=(kt == 0),
                                stop=(kt == FT - 1),
                            )
                        # relu(x + b1) on DVE: (x add b1) max 0
                        nc.vector.tensor_scalar(
                            sb_hT[:, ht2, :], ps, sb_b1[:, ht2 : ht2 + 1], 0.0,
                            op0=ALU.add, op1=ALU.max,
                        )

            if MAX_PHASE >= 5:
                # ============ Phase 5: FFN2 + LN2 + out ============
                with (
                    tc.tile_pool(name="w2_p", bufs=1) as w2_p,
                    tc.tile_pool(name="ln2p", bufs=2) as ln2p,
                    tc.tile_pool(name="outp", bufs=3) as outp,
                    tc.tile_pool(name="ps_f2", bufs=4, space="PSUM") as ps_f2,
                ):
                    w2_t = []
                    for kt in range(HT):
                        wt = w2_p.tile([128, DIM], bf16, tag=f"w2_{kt}")
                        nc.sync.dma_start(
                            out=wt, in_=d_w2[kt * 128 : (kt + 1) * 128, :]
                        )
                        w2_t.append(wt)
                    out_r = d_out[:].rearrange("(t p) d -> p t d", p=128)
                    for tt in range(TT):
                        f2pre = ln2p.tile([128, DIM], f32, tag="f2pre")
                        for nh in range(2):
                            ps = ps_f2.tile([128, 384], f32, tag="f2")
                            for kt in range(HT):
                                nc.tensor.matmul(
                                    ps,
                                    sb_hT[:, kt, tt * 128 : (tt + 1) * 128],
                                    w2_t[kt][:, nh * 384 : (nh + 1) * 384],
                                    start=(kt == 0),
                                    stop=(kt == HT - 1),
                                )
                            nc.vector.scalar_tensor_tensor(
                                out=f2pre[:, nh * 384 : (nh + 1) * 384],
                                in0=ps,
                                scalar=1.0,
                                in1=b2_bc[:, nh * 384 : (nh + 1) * 384],
                                op0=ALU.mult,
                                op1=ALU.add,
                            )
                        l1b = ln2p.tile([128, DIM], f32, tag="l1b")
                        nc.vector.tensor_add(l1b, sb_l1[:, tt, :], bb2_bc)
                        o_sb = outp.tile([128, DIM], f32, tag="osb")
                        layer_norm_to(o_sb[:], f2pre[:], g2_bc, l1b, ln2p)
                        nc.sync.dma_start(out=out_r[:, tt, :], in_=o_sb)

    return nc


def _get_nc(finalized=True):
    if "nc" not in _CACHE:
        _CACHE["nc"] = _build_program()
    nc = _CACHE["nc"]
    if finalized and not nc.is_finalized():
        nc.finalize()
    return nc


def make_in_maps(inputs: dict) -> list:
    x = np.asarray(inputs["x_n"], np.float32).reshape(B, S, DIM)
    mask = np.asarray(inputs["mask"]).reshape(B, S)
    w = {
        k: np.ascontiguousarray(np.asarray(inputs[k], np.float32).astype(BF16))
        for k in ("wq", "wk", "wv", "wo", "w1", "w2")
    }
    vecs = {
        "bq": inputs["bq"], "bk": inputs["bk"], "bv": inputs["bv"],
        "bo": inputs["bo"], "b1": inputs["b1"], "b2": inputs["b2"],
        "g1": inputs["ln1_g"], "bb1": inputs["ln1_b"],
        "g2": inputs["ln2_g"], "bb2": inputs["ln2_b"],
    }
    vecs = {k: np.ascontiguousarray(np.asarray(v, np.float32)) for k, v in vecs.items()}
    in_maps = []
    for c in range(N_CORES):
        b, blk = c // NBLK, c % NBLK
        xb = x[b]
        xT = None if USE_AG else np.ascontiguousarray(xb.T.astype(BF16))
        xblk = np.ascontiguousarray(xb[blk * BLK : (blk + 1) * BLK])
        xTb = np.ascontiguousarray(xblk.T.astype(BF16))
        msc = (mask[b].astype(np.float32) != 0).astype(np.float32) * ISCALE
        m = {"xTb": xTb, "xb": xblk, "msc": msc}
        if not USE_AG:
            m["xT"] = xT
        m.update(w)
        m.update(vecs)
        in_maps.append(m)
    return in_maps


def assemble(per_core_out: list) -> np.ndarray:
    blocks = [np.asarray(o, np.float32) for o in per_core_out]
    full = np.concatenate(blocks, axis=0).reshape(B, S, DIM)
    return full


def kernel(**inputs) -> np.ndarray:
    from concourse.bass_utils import run_bass_kernel_spmd

    nc = _get_nc()
    in_maps = make_in_maps(inputs)
    res = run_bass_kernel_spmd(nc, in_maps, list(range(N_CORES)))
    return assemble([r["out"] for r in res.results])



# revision 3
# speedup vs baseline: 1.1181x; 1.1181x over previous
"""Trainium2 Bass kernel for a dense transformer encoder layer.

Model (faithful to the oracle):
  q,k,v = x@wq+bq, x@wk+bk, x@wv+bv          (12 heads, dk=64, DIM=768)
  scores = q@k^T / sqrt(768)  (note: sqrt(dim_model), not sqrt(dk))
  scores[mask==0] = 1e-11  (NOT -inf; masked keys still contribute ~1/Z)
  attn = softmax(scores); z = attn@v; o = z@wo+bo
  l1 = x + LN(o);  ffn = relu(l1@w1+b1)@w2+b2;  out = l1 + LN(ffn)

Sharding: 4096 tokens (B=2,S=2048) split 8 ways -> 512 tokens/core.
Cores 0-3 own batch 0, cores 4-7 batch 1. K/V are computed for the
core's whole batch (redundantly within each 4-core group) so attention
needs no collectives.

Softmax trick: scores are built k-major (scoresT [kpos, q]) so the
mask (per-k) is a per-partition scalar; exp(mask_p/sqrt(768) * s) on
the scalar engine applies scale+mask+exp in a single pass (masked rows
give exp(0)=1.0 == fp32(exp(1e-11))). The denominator comes from a
ones column appended to V (attn@v with M=65); normalization happens
after attn@v via a rank-1 matmul broadcast of 1/sum.
"""

import math
import os
import sys

import numpy as np

for _p in ("/opt/trn_rl_repo", os.path.expanduser("~/.axon_site/_ro/trn_rl_repo")):
    if os.path.isdir(_p) and _p not in sys.path:
        sys.path.insert(0, _p)

import ml_dtypes  # noqa: E402

BF16 = ml_dtypes.bfloat16

DIM = 768
HEADS = 12
DK = 64
HID = 4 * DIM  # 3072
B, S = 2, 2048
N_CORES = 8
BLK = 512            # tokens per core
NBLK = S // BLK      # 4 blocks per batch
EPS = 1e-5
ISCALE = 1.0 / math.sqrt(DIM)

_CACHE: dict = {}
MAX_PHASE = int(os.environ.get("BASS_KERNEL_PHASES", "5"))
USE_AG = os.environ.get("BASS_KERNEL_AG", "1") == "1"


def _build_program():
    import concourse.bass as bass
    import concourse.mybir as mybir
    import concourse.tile as tile
    from concourse import bacc
    from concourse.masks import make_identity

    f32 = mybir.dt.float32
    bf16 = mybir.dt.bfloat16
    AF = mybir.ActivationFunctionType
    ALU = mybir.AluOpType
    AX = mybir.AxisListType

    nc = bacc.Bacc()

    # ---- per-core DRAM I/O ----
    if not USE_AG:
        d_xT = nc.dram_tensor("xT", [DIM, S], bf16, kind="ExternalInput")
    d_xTb = nc.dram_tensor("xTb", [DIM, BLK], bf16, kind="ExternalInput")
    d_xb = nc.dram_tensor("xb", [BLK, DIM], f32, kind="ExternalInput")
    d_msc = nc.dram_tensor("msc", [S], f32, kind="ExternalInput")
    d_wq = nc.dram_tensor("wq", [DIM, DIM], bf16, kind="ExternalInput")
    d_wk = nc.dram_tensor("wk", [DIM, DIM], bf16, kind="ExternalInput")
    d_wv = nc.dram_tensor("wv", [DIM, DIM], bf16, kind="ExternalInput")
    d_wo = nc.dram_tensor("wo", [DIM, DIM], bf16, kind="ExternalInput")
    d_w1 = nc.dram_tensor("w1", [DIM, HID], bf16, kind="ExternalInput")
    d_w2 = nc.dram_tensor("w2", [HID, DIM], bf16, kind="ExternalInput")
    d_bq = nc.dram_tensor("bq", [DIM], f32, kind="ExternalInput")
    d_bk = nc.dram_tensor("bk", [DIM], f32, kind="ExternalInput")
    d_bv = nc.dram_tensor("bv", [DIM], f32, kind="ExternalInput")
    d_bo = nc.dram_tensor("bo", [DIM], f32, kind="ExternalInput")
    d_b1 = nc.dram_tensor("b1", [HID], f32, kind="ExternalInput")
    d_b2 = nc.dram_tensor("b2", [DIM], f32, kind="ExternalInput")
    d_g1 = nc.dram_tensor("g1", [DIM], f32, kind="ExternalInput")
    d_bb1 = nc.dram_tensor("bb1", [DIM], f32, kind="ExternalInput")
    d_g2 = nc.dram_tensor("g2", [DIM], f32, kind="ExternalInput")
    d_bb2 = nc.dram_tensor("bb2", [DIM], f32, kind="ExternalInput")
    d_out = nc.dram_tensor("out", [BLK, DIM], f32, kind="ExternalOutput")
    if USE_AG:
        d_kb = nc.dram_tensor("k_bounce", [DIM, BLK], bf16)
        d_ks = nc.dram_tensor("k_shared", [NBLK * DIM, BLK], bf16)
        d_vb = nc.dram_tensor("v_bounce", [BLK, HEADS * (DK + 1)], bf16)
        d_vs = nc.dram_tensor("v_shared", [S, HEADS * (DK + 1)], bf16)
        RG = [[0, 1, 2, 3], [4, 5, 6, 7]]

    FT = DIM // 128   # 6 feature tiles
    TT = BLK // 128   # 4 token tiles per core block
    ST = S // 128     # 16 token tiles per batch
    HT = HID // 128   # 24 hidden tiles

    def bcast_ap(handle, n=128):
        ap = handle[:]
        return bass.AP(tensor=ap.tensor, offset=ap.offset, ap=[[0, n]] + list(ap.ap))

    with tile.TileContext(nc) as tc:
        with (
            tc.tile_pool(name="const", bufs=1) as const,
            tc.tile_pool(name="bigres", bufs=1) as big,
        ):
            # ---------- constants ----------
            sb_msc = const.tile([128, ST], f32)
            nc.sync.dma_start(out=sb_msc, in_=d_msc[:].rearrange("(t p) -> p t", p=128))
            sb_bq = const.tile([128, FT], f32)
            nc.sync.dma_start(out=sb_bq, in_=d_bq[:].rearrange("(t p) -> p t", p=128))
            sb_bk = const.tile([128, FT], f32)
            nc.sync.dma_start(out=sb_bk, in_=d_bk[:].rearrange("(t p) -> p t", p=128))
            sb_b1 = const.tile([128, HT], f32)
            nc.sync.dma_start(out=sb_b1, in_=d_b1[:].rearrange("(t p) -> p t", p=128))
            bv_bc = const.tile([128, DIM], f32)
            nc.gpsimd.dma_start(out=bv_bc, in_=bcast_ap(d_bv))
            bo_bc = const.tile([128, DIM], f32)
            nc.gpsimd.dma_start(out=bo_bc, in_=bcast_ap(d_bo))
            b2_bc = const.tile([128, DIM], f32)
            nc.gpsimd.dma_start(out=b2_bc, in_=bcast_ap(d_b2))
            g1_bc = const.tile([128, DIM], f32)
            nc.gpsimd.dma_start(out=g1_bc, in_=bcast_ap(d_g1))
            bb1_bc = const.tile([128, DIM], f32)
            nc.gpsimd.dma_start(out=bb1_bc, in_=bcast_ap(d_bb1))
            g2_bc = const.tile([128, DIM], f32)
            nc.gpsimd.dma_start(out=g2_bc, in_=bcast_ap(d_g2))
            bb2_bc = const.tile([128, DIM], f32)
            nc.gpsimd.dma_start(out=bb2_bc, in_=bcast_ap(d_bb2))
            ident = const.tile([128, 128], f32)
            make_identity(nc, ident[:])
            # selector for broadcasting the two per-half softmax denominators
            # into partitions 0-63 / 64-127 with a single K=2 matmul
            sel2 = const.tile([2, 128], f32)
            nc.vector.memset(sel2, 0.0)
            nc.vector.memset(sel2[0:1, 0:64], 1.0)
            nc.vector.memset(sel2[1:2, 64:128], 1.0)
            eps_t = const.tile([128, 1], f32)
            nc.vector.memset(eps_t, EPS)

            # ---------- persistent activations ----------
            sb_xblk = big.tile([128, TT, DIM], f32)  # residual x
            sb_l1 = big.tile([128, TT, DIM], f32)

            nc.sync.dma_start(
                out=sb_xblk, in_=d_xb[:].rearrange("(t p) d -> p t d", p=128)
            )

            # attention-scoped residents (freed before the FFN phases)
            attn_res_cm = tc.tile_pool(name="attn_res", bufs=1)
            attn_res = attn_res_cm.__enter__()
            sb_K = attn_res.tile([128, FT, NBLK, BLK], bf16)  # K^T, feat-major
            sb_Q = attn_res.tile([128, FT, BLK], bf16)  # Q^T, feat-major
            sb_V = attn_res.tile([128, ST, HEADS, DK + 1], bf16)  # V + ones col
            sb_zT = attn_res.tile([128, FT, BLK], bf16)  # z^T normalized

            # ============ Phase 1: QKV projections ============
            with (
                tc.tile_pool(name="xw", bufs=1) as xw,
                tc.tile_pool(name="ps1", bufs=4, space="PSUM") as ps1,
                tc.tile_pool(name="ps1v", bufs=4, space="PSUM") as ps1v,
            ):
                if not USE_AG:
                    sb_xT = xw.tile([128, FT, S], bf16)
                    nc.sync.dma_start(
                        out=sb_xT, in_=d_xT[:].rearrange("(t p) n -> p t n", p=128)
                    )
                sb_xTb = xw.tile([128, FT, BLK], bf16)
                nc.sync.dma_start(
                    out=sb_xTb, in_=d_xTb[:].rearrange("(t p) n -> p t n", p=128)
                )
                w_q = xw.tile([128, FT, DIM], bf16)
                nc.sync.dma_start(
                    out=w_q, in_=d_wq[:].rearrange("(t p) o -> p t o", p=128)
                )
                w_k = xw.tile([128, FT, DIM], bf16)
                nc.sync.dma_start(
                    out=w_k, in_=d_wk[:].rearrange("(t p) o -> p t o", p=128)
                )
                w_v = xw.tile([128, FT, DIM], bf16)
                nc.sync.dma_start(
                    out=w_v, in_=d_wv[:].rearrange("(t p) o -> p t o", p=128)
                )

                if USE_AG:
                    # K^T feat-major for the own block only -> bounce -> AG
                    kstage = xw.tile([128, FT, BLK], bf16, tag="kstage")
                    for ft in range(FT):
                        ps = ps1.tile([128, 512], f32, tag="p")
                        for kt in range(FT):
                            nc.tensor.matmul(
                                ps,
                                w_k[:, kt, ft * 128 : (ft + 1) * 128],
                                sb_xTb[:, kt, :],
                                start=(kt == 0),
                                stop=(kt == FT - 1),
                            )
                        nc.vector.tensor_scalar_add(
                            kstage[:, ft, :], ps, sb_bk[:, ft : ft + 1]
                        )
                    nc.sync.dma_start(
                        out=d_kb[:].rearrange("(t p) n -> p t n", p=128), in_=kstage
                    )
                else:
                    # K^T feat-major over the whole batch (replicated)
                    for ft in range(FT):
                        for nt in range(S // 512):
                            ps = ps1.tile([128, 512], f32, tag="p")
                            for kt in range(FT):
                                nc.tensor.matmul(
                                    ps,
                                    w_k[:, kt, ft * 128 : (ft + 1) * 128],
                                    sb_xT[:, kt, nt * 512 : (nt + 1) * 512],
                                    start=(kt == 0),
                                    stop=(kt == FT - 1),
                                )
                            nc.vector.tensor_scalar_add(
                                sb_K[:, ft, nt, :], ps, sb_bk[:, ft : ft + 1]
                            )
                # Q^T feat-major for the core's block
                for ft in range(FT):
                    ps = ps1.tile([128, 512], f32, tag="p")
                    for kt in range(FT):
                        nc.tensor.matmul(
                            ps,
                            w_q[:, kt, ft * 128 : (ft + 1) * 128],
                            sb_xTb[:, kt, :],
                            start=(kt == 0),
                            stop=(kt == FT - 1),
                        )
                    nc.vector.tensor_scalar_add(
                        sb_Q[:, ft, :], ps, sb_bq[:, ft : ft + 1]
                    )
                if USE_AG:
                    # V tok-major for the own block -> bounce -> AG
                    vstage = xw.tile([128, TT, HEADS, DK + 1], bf16, tag="vstage")
                    nc.vector.memset(vstage[:, :, :, DK : DK + 1], 1.0)
                    for tt in range(TT):
                        for nh in range(2):
                            ps = ps1v.tile([128, 384], f32, tag="vp")
                            for kt in range(FT):
                                nc.tensor.matmul(
                                    ps,
                                    sb_xTb[:, kt, tt * 128 : (tt + 1) * 128],
                                    w_v[:, kt, nh * 384 : (nh + 1) * 384],
                                    start=(kt == 0),
                                    stop=(kt == FT - 1),
                                )
                            nc.vector.scalar_tensor_tensor(
                                out=vstage[:, tt, nh * 6 : (nh + 1) * 6, 0:DK],
                                in0=ps[:].rearrange("p (h d) -> p h d", d=DK),
                                scalar=1.0,
                                in1=bv_bc[:, nh * 384 : (nh + 1) * 384].rearrange(
                                    "p (h d) -> p h d", d=DK
                                ),
                                op0=ALU.mult,
                                op1=ALU.add,
                            )
                    nc.sync.dma_start(
                        out=d_vb[:].rearrange("(t p) (h d) -> p t h d", p=128, d=DK + 1),
                        in_=vstage,
                    )
                    # AllGather K and V across the 4-core batch group
                    nc.gpsimd.collective_compute(
                        "AllGather", ALU.bypass, replica_groups=RG,
                        ins=[d_kb[:]], outs=[d_ks[:]],
                    )
                    nc.gpsimd.collective_compute(
                        "AllGather", ALU.bypass, replica_groups=RG,
                        ins=[d_vb[:]], outs=[d_vs[:]],
                    )
                    for b in range(NBLK):
                        nc.sync.dma_start(
                            out=sb_K[:, :, b, :],
                            in_=d_ks[b * DIM : (b + 1) * DIM, :].rearrange(
                                "(t p) n -> p t n", p=128
                            ),
                        )
                    nc.sync.dma_start(
                        out=sb_V,
                        in_=d_vs[:].rearrange(
                            "(t p) (h d) -> p t h d", p=128, d=DK + 1
                        ),
                    )
                else:
                    # V tok-major over the whole batch, laid out [tok, head, dk+1]
                    nc.vector.memset(sb_V[:, :, :, DK : DK + 1], 1.0)
                    for nh in range(2):
                        for tt in range(ST):
                            ps = ps1v.tile([128, 384], f32, tag="vp")
                            for kt in range(FT):
                                nc.tensor.matmul(
                                    ps,
                                    sb_xT[:, kt, tt * 128 : (tt + 1) * 128],
                                    w_v[:, kt, nh * 384 : (nh + 1) * 384],
                                    start=(kt == 0),
                                    stop=(kt == FT - 1),
                                )
                            nc.vector.scalar_tensor_tensor(
                                out=sb_V[:, tt, nh * 6 : (nh + 1) * 6, 0:DK],
                                in0=ps[:].rearrange("p (h d) -> p h d", d=DK),
                                scalar=1.0,
                                in1=bv_bc[:, nh * 384 : (nh + 1) * 384].rearrange(
                                    "p (h d) -> p h d", d=DK
                                ),
                                op0=ALU.mult,
                                op1=ALU.add,
                            )

            if MAX_PHASE >= 2:
                # ============ Phase 2: attention ============
                with (
                    tc.tile_pool(name="expp", bufs=64) as expp,
                    tc.tile_pool(name="attsm", bufs=2) as attsm,
                    tc.tile_pool(name="ps_sc", bufs=4, space="PSUM") as ps_sc,
                    tc.tile_pool(name="ps_z", bufs=2, space="PSUM") as ps_z,
                    tc.tile_pool(name="ps_rb", bufs=1, space="PSUM") as ps_rb,
                ):
                    for hp in range(HEADS // 2):
                        ht = hp
                        # interleave the two heads of a pair kt-by-kt: their
                        # K=64 matmuls sit in disjoint PE row groups (0-63 /
                        # 64-127) so the hardware overlaps adjacent pairs.
                        ets = ([], [])
                        for kt2 in range(ST):
                            for half in (0, 1):
                                ho = half * 64
                                ps = ps_sc.tile([128, BLK], f32, tag="sc")
                                nc.tensor.matmul(
                                    ps,
                                    sb_K[ho : ho + 64, ht, kt2 // 4, (kt2 % 4) * 128 : (kt2 % 4) * 128 + 128],
                                    sb_Q[ho : ho + 64, ht, :],
                                    start=True,
                                    stop=True,
                                )
                                et = expp.tile([128, BLK], bf16, tag="exp")
                                nc.scalar.activation(
                                    et, ps, AF.Exp, scale=sb_msc[:, kt2 : kt2 + 1]
                                )
                                ets[half].append(et)
                        zps = []
                        for half in (0, 1):
                            h = 2 * hp + half
                            zp = ps_z.tile([DK + 1, BLK], f32, tag=f"z{half}")
                            for kt2 in range(ST):
                                nc.tensor.matmul(
                                    zp,
                                    sb_V[:, kt2, h, :],
                                    ets[half][kt2],
                                    start=(kt2 == 0),
                                    stop=(kt2 == ST - 1),
                                )
                            zps.append(zp)
                        # both halves' denominators -> [2, BLK], one reciprocal,
                        # one K=2 selector matmul broadcasts them to 64 rows each
                        rs2 = attsm.tile([2, BLK], f32, tag="rsum")
                        nc.vector.tensor_copy(rs2[0:1, :], zps[0][DK : DK + 1, :])
                        nc.vector.tensor_copy(rs2[1:2, :], zps[1][DK : DK + 1, :])
                        rr2 = attsm.tile([2, BLK], f32, tag="rrec")
                        nc.vector.reciprocal(rr2, rs2)
                        rbp = ps_rb.tile([128, BLK], f32, tag="rb")
                        nc.tensor.matmul(rbp, sel2[:], rr2, start=True, stop=True)
                        rb = attsm.tile([128, BLK], f32, tag="rbs")
                        nc.vector.tensor_copy(rb, rbp)
                        nc.vector.tensor_mul(
                            sb_zT[0:64, ht, :], zps[0][0:DK, :], rb[0:64, :]
                        )
                        nc.vector.tensor_mul(
                            sb_zT[64:128, ht, :], zps[1][0:DK, :], rb[64:128, :]
                        )

            if MAX_PHASE >= 3:
                # ============ Phase 3: O proj + LN1 (+residual) ============
                def layer_norm_to(out_ap, x_ap, g_bc_t, resid_ap, pool):
                    s = pool.tile([128, 1], f32, tag="ln_s")
                    nc.vector.tensor_reduce(s, x_ap, axis=AX.X, op=ALU.add)
                    mean = pool.tile([128, 1], f32, tag="ln_m")
                    nc.scalar.mul(mean, s, 1.0 / DIM)
                    xc = pool.tile([128, DIM], f32, tag="ln_xc")
                    nc.vector.tensor_scalar(xc, x_ap, mean, None, op0=ALU.subtract)
                    junk = pool.tile([128, DIM], f32, tag="ln_j")
                    var = pool.tile([128, 1], f32, tag="ln_v")
                    # (tensor_tensor_reduce crashes the device on this runtime;
                    # scalar_tensor_tensor with accum_out works)
                    nc.vector.scalar_tensor_tensor(
                        out=junk, in0=xc, scalar=1.0, in1=xc,
                        op0=ALU.mult, op1=ALU.mult, accum_out=var,
                    )
                    nc.vector.tensor_scalar_mul(var, var, 1.0 / DIM)
                    sd = pool.tile([128, 1], f32, tag="ln_sd")
                    nc.scalar.activation(sd, var, AF.Sqrt, bias=eps_t[:])
                    rstd = pool.tile([128, 1], f32, tag="ln_r")
                    nc.vector.reciprocal(rstd, sd)
                    t = pool.tile([128, DIM], f32, tag="ln_t")
                    nc.vector.tensor_scalar(t, xc, rstd, None, op0=ALU.mult)
                    tg = pool.tile([128, DIM], f32, tag="ln_tg")
                    nc.vector.tensor_mul(tg, t, g_bc_t)
                    nc.vector.tensor_add(out_ap, tg, resid_ap)

                with (
                    tc.tile_pool(name="wo_p", bufs=1) as wo_p,
                    tc.tile_pool(name="ln1p", bufs=2) as ln1p,
                    tc.tile_pool(name="ps_o", bufs=4, space="PSUM") as ps_o,
                ):
                    w_o = wo_p.tile([128, FT, DIM], bf16)
                    nc.sync.dma_start(
                        out=w_o, in_=d_wo[:].rearrange("(t p) o -> p t o", p=128)
                    )
                    for tt in range(TT):
                        l1pre = ln1p.tile([128, DIM], f32, tag="l1pre")
                        for nh in range(2):
                            ps = ps_o.tile([128, 384], f32, tag="op")
                            for kt in range(FT):
                                nc.tensor.matmul(
                                    ps,
                                    sb_zT[:, kt, tt * 128 : (tt + 1) * 128],
                                    w_o[:, kt, nh * 384 : (nh + 1) * 384],
                                    start=(kt == 0),
                                    stop=(kt == FT - 1),
                                )
                            nc.vector.scalar_tensor_tensor(
                                out=l1pre[:, nh * 384 : (nh + 1) * 384],
                                in0=ps,
                                scalar=1.0,
                                in1=bo_bc[:, nh * 384 : (nh + 1) * 384],
                                op0=ALU.mult,
                                op1=ALU.add,
                            )
                        xb1 = ln1p.tile([128, DIM], f32, tag="xb1")
                        nc.vector.tensor_add(xb1, sb_xblk[:, tt, :], bb1_bc)
                        layer_norm_to(sb_l1[:, tt, :], l1pre[:], g1_bc, xb1, ln1p)

            attn_res_cm.__exit__(None, None, None)
            sb_hT = big.tile([128, HT, BLK], bf16)  # relu(ffn1)^T, hid-major

            if MAX_PHASE >= 4:
                # ============ Phase 4: transpose l1, FFN1 ============
                with (
                    tc.tile_pool(name="w1_p", bufs=1) as w1_p,
                    tc.tile_pool(name="l1t_p", bufs=1) as l1t_p,
                    tc.tile_pool(name="ps_t", bufs=2, space="PSUM") as ps_t,
                    tc.tile_pool(name="ps_f1", bufs=4, space="PSUM") as ps_f1,
                ):
                    w1_t = []
                    for kt in range(FT):
                        wt = w1_p.tile([128, HID], bf16, tag=f"w1_{kt}")
                        nc.sync.dma_start(
                            out=wt, in_=d_w1[kt * 128 : (kt + 1) * 128, :]
                        )
                        w1_t.append(wt)
                    sb_l1T = l1t_p.tile([128, FT, BLK], bf16)
                    for ft in range(FT):
                        for tt in range(TT):
                            pst = ps_t.tile([128, 128], f32, tag="tp")
                            nc.tensor.transpose(
                                pst, sb_l1[:, tt, ft * 128 : (ft + 1) * 128], ident[:]
                            )
                            nc.scalar.copy(
                                sb_l1T[:, ft, tt * 128 : (tt + 1) * 128], pst
                            )
                    for ht2 in range(HT):
                        ps = ps_f1.tile([128, BLK], f32, tag="f1")
                        for kt in range(FT):
                            nc.tensor.matmul(
                                ps,
                                w1_t[kt][:, ht2 * 128 : (ht2 + 1) * 128],
                                sb_l1T[:, kt, :],
                                start=(kt == 0),
                                stop=(kt == FT - 1),
                            )
                        # relu(x + b1) on DVE: (x add b1) max 0
                        nc.vector.tensor_scalar(
                            sb_hT[:, ht2, :], ps, sb_b1[:, ht2 : ht2 + 1], 0.0,
                            op0=ALU.add, op1=ALU.max,
                        )

            if MAX_PHASE >= 5:
                # ============ Phase 5: FFN2 + LN2 + out ============
                with (
                    tc.tile_pool(name="w2_p", bufs=1) as w2_p,
                    tc.tile_pool(name="ln2p", bufs=2) as ln2p,
                    tc.tile_pool(name="outp", bufs=3) as outp,
                    tc.tile_pool(name="ps_f2", bufs=4, space="PSUM") as ps_f2,
                ):
                    w2_t = []
                    for kt in range(HT):
                        wt = w2_p.tile([128, DIM], bf16, tag=f"w2_{kt}")
                        nc.sync.dma_start(
                            out=wt, in_=d_w2[kt * 128 : (kt + 1) * 128, :]
                        )
                        w2_t.append(wt)
                    out_r = d_out[:].rearrange("(t p) d -> p t d", p=128)
                    for tt in range(TT):
                        f2pre = ln2p.tile([128, DIM], f32, tag="f2pre")
                        for nh in range(2):
                            ps = ps_f2.tile([128, 384], f32, tag="f2")
                            for kt in range(HT):
                                nc.tensor.matmul(
                                    ps,
                                    sb_hT[:, kt, tt * 128 : (tt + 1) * 128],
                                    w2_t[kt][:, nh * 384 : (nh + 1) * 384],
                                    start=(kt == 0),
                                    stop=(kt == HT - 1),
                                )
                            nc.vector.scalar_tensor_tensor(
                                out=f2pre[:, nh * 384 : (nh + 1) * 384],
                                in0=ps,
                                scalar=1.0,
                                in1=b2_bc[:, nh * 384 : (nh + 1) * 384],
                                op0=ALU.mult,
                                op1=ALU.add,
                            )
                        l1b = ln2p.tile([128, DIM], f32, tag="l1b")
                        nc.vector.tensor_add(l1b, sb_l1[:, tt, :], bb2_bc)
                        o_sb = outp.tile([128, DIM], f32, tag="osb")
                        layer_norm_to(o_sb[:], f2pre[:], g2_bc, l1b, ln2p)
                        nc.sync.dma_start(out=out_r[:, tt, :], in_=o_sb)

    return nc


def _get_nc(finalized=True):
    if "nc" not in _CACHE:
        _CACHE["nc"] = _build_program()
    nc = _CACHE["nc"]
    if finalized and not nc.is_finalized():
        nc.finalize()
    return nc


def make_in_maps(inputs: dict) -> list:
    x = np.asarray(inputs["x_n"], np.float32).reshape(B, S, DIM)
    mask = np.asarray(inputs["mask"]).reshape(B, S)
    w = {
        k: np.ascontiguousarray(np.asarray(inputs[k], np.float32).astype(BF16))
        for k in ("wq", "wk", "wv", "wo", "w1", "w2")
    }
    vecs = {
        "bq": inputs["bq"], "bk": inputs["bk"], "bv": inputs["bv"],
        "bo": inputs["bo"], "b1": inputs["b1"], "b2": inputs["b2"],
        "g1": inputs["ln1_g"], "bb1": inputs["ln1_b"],
        "g2": inputs["ln2_g"], "bb2": inputs["ln2_b"],
    }
    vecs = {k: np.ascontiguousarray(np.asarray(v, np.float32)) for k, v in vecs.items()}
    in_maps = []
    for c in range(N_CORES):
        b, blk = c // NBLK, c % NBLK
        xb = x[b]
        xT = None if USE_AG else np.ascontiguousarray(xb.T.astype(BF16))
        xblk = np.ascontiguousarray(xb[blk * BLK : (blk + 1) * BLK])
        xTb = np.ascontiguousarray(xblk.T.astype(BF16))
        msc = (mask[b].astype(np.float32) != 0).astype(np.float32) * ISCALE
        m = {"xTb": xTb, "xb": xblk, "msc": msc}
        if not USE_AG:
            m["xT"] = xT
        m.update(w)
        m.update(vecs)
        in_maps.append(m)
    return in_maps


def assemble(per_core_out: list) -> np.ndarray:
    blocks = [np.asarray(o, np.float32) for o in per_core_out]
    full = np.concatenate(blocks, axis=0).reshape(B, S, DIM)
    return full


def kernel(**inputs) -> np.ndarray:
    from concourse.bass_utils import run_bass_kernel_spmd

    nc = _get_nc()
    in_maps = make_in_maps(inputs)
    res = run_bass_kernel_spmd(nc, in_maps, list(range(N_CORES)))
    return assemble([r["out"] for r in res.results])



# revision 13
# speedup vs baseline: 1.1988x; 1.0722x over previous
"""Trainium2 Bass kernel for a dense transformer encoder layer.

Model (faithful to the oracle):
  q,k,v = x@wq+bq, x@wk+bk, x@wv+bv          (12 heads, dk=64, DIM=768)
  scores = q@k^T / sqrt(768)  (note: sqrt(dim_model), not sqrt(dk))
  scores[mask==0] = 1e-11  (NOT -inf; masked keys still contribute ~1/Z)
  attn = softmax(scores); z = attn@v; o = z@wo+bo
  l1 = x + LN(o);  ffn = relu(l1@w1+b1)@w2+b2;  out = l1 + LN(ffn)

Sharding: 4096 tokens (B=2,S=2048) split 8 ways -> 512 tokens/core.
Cores 0-3 own batch 0, cores 4-7 batch 1. K/V are computed for the
core's whole batch (redundantly within each 4-core group) so attention
needs no collectives.

Softmax trick: scores are built k-major (scoresT [kpos, q]) so the
mask (per-k) is a per-partition scalar; exp(mask_p/sqrt(768) * s) on
the scalar engine applies scale+mask+exp in a single pass (masked rows
give exp(0)=1.0 == fp32(exp(1e-11))). The denominator comes from a
ones column appended to V (attn@v with M=65); normalization happens
after attn@v via a rank-1 matmul broadcast of 1/sum.
"""

import math
import os
import sys

import numpy as np

for _p in ("/opt/trn_rl_repo", os.path.expanduser("~/.axon_site/_ro/trn_rl_repo")):
    if os.path.isdir(_p) and _p not in sys.path:
        sys.path.insert(0, _p)

import ml_dtypes  # noqa: E402

BF16 = ml_dtypes.bfloat16

DIM = 768
HEADS = 12
DK = 64
HID = 4 * DIM  # 3072
B, S = 2, 2048
N_CORES = 8
BLK = 512            # tokens per core
NBLK = S // BLK      # 4 blocks per batch
EPS = 1e-5
ISCALE = 1.0 / math.sqrt(DIM)

_CACHE: dict = {}
MAX_PHASE = int(os.environ.get("BASS_KERNEL_PHASES", "5"))
USE_AG = os.environ.get("BASS_KERNEL_AG", "0") == "1"


def _build_program():
    import concourse.bass as bass
    import concourse.mybir as mybir
    import concourse.tile as tile
    from concourse import bacc
    from concourse.masks import make_identity

    f32 = mybir.dt.float32
    bf16 = mybir.dt.bfloat16
    AF = mybir.ActivationFunctionType
    ALU = mybir.AluOpType
    AX = mybir.AxisListType

    nc = bacc.Bacc()

    # ---- per-core DRAM I/O ----
    if not USE_AG:
        d_xT = nc.dram_tensor("xT", [DIM, S], bf16, kind="ExternalInput")
    d_xTb = nc.dram_tensor("xTb", [DIM, BLK], bf16, kind="ExternalInput")
    d_xb = nc.dram_tensor("xb", [BLK, DIM], f32, kind="ExternalInput")
    d_msc = nc.dram_tensor("msc", [S], f32, kind="ExternalInput")
    d_wq = nc.dram_tensor("wq", [DIM, DIM], bf16, kind="ExternalInput")
    d_wk = nc.dram_tensor("wk", [DIM, DIM], bf16, kind="ExternalInput")
    d_wv = nc.dram_tensor("wv", [DIM, DIM], bf16, kind="ExternalInput")
    d_wo = nc.dram_tensor("wo", [DIM, DIM], bf16, kind="ExternalInput")
    d_w1 = nc.dram_tensor("w1", [DIM, HID], bf16, kind="ExternalInput")
    d_w2 = nc.dram_tensor("w2", [HID, DIM], bf16, kind="ExternalInput")
    d_bq = nc.dram_tensor("bq", [DIM], f32, kind="ExternalInput")
    d_bk = nc.dram_tensor("bk", [DIM], f32, kind="ExternalInput")
    d_bv = nc.dram_tensor("bv", [DIM], f32, kind="ExternalInput")
    d_bo = nc.dram_tensor("bo", [DIM], f32, kind="ExternalInput")
    d_b1 = nc.dram_tensor("b1", [HID], f32, kind="ExternalInput")
    d_b2 = nc.dram_tensor("b2", [DIM], f32, kind="ExternalInput")
    d_g1 = nc.dram_tensor("g1", [DIM], f32, kind="ExternalInput")
    d_bb1 = nc.dram_tensor("bb1", [DIM], f32, kind="ExternalInput")
    d_g2 = nc.dram_tensor("g2", [DIM], f32, kind="ExternalInput")
    d_bb2 = nc.dram_tensor("bb2", [DIM], f32, kind="ExternalInput")
    d_out = nc.dram_tensor("out", [BLK, DIM], f32, kind="ExternalOutput")
    if USE_AG:
        d_kb = nc.dram_tensor("k_bounce", [DIM, BLK], bf16)
        d_ks = nc.dram_tensor("k_shared", [NBLK * DIM, BLK], bf16)
        d_vb = nc.dram_tensor("v_bounce", [BLK, HEADS * (DK + 1)], bf16)
        d_vs = nc.dram_tensor("v_shared", [S, HEADS * (DK + 1)], bf16)
        RG = [[0, 1, 2, 3], [4, 5, 6, 7]]

    FT = DIM // 128   # 6 feature tiles
    TT = BLK // 128   # 4 token tiles per core block
    ST = S // 128     # 16 token tiles per batch
    HT = HID // 128   # 24 hidden tiles

    def bcast_ap(handle, n=128):
        ap = handle[:]
        return bass.AP(tensor=ap.tensor, offset=ap.offset, ap=[[0, n]] + list(ap.ap))

    with tile.TileContext(nc) as tc:
        with (
            tc.tile_pool(name="const", bufs=1) as const,
            tc.tile_pool(name="bigres", bufs=1) as big,
        ):
            # ---------- constants ----------
            sb_msc = const.tile([128, ST], f32)
            nc.sync.dma_start(out=sb_msc, in_=d_msc[:].rearrange("(t p) -> p t", p=128))
            sb_bq = const.tile([128, FT], f32)
            nc.sync.dma_start(out=sb_bq, in_=d_bq[:].rearrange("(t p) -> p t", p=128))
            sb_bk = const.tile([128, FT], f32)
            nc.sync.dma_start(out=sb_bk, in_=d_bk[:].rearrange("(t p) -> p t", p=128))
            sb_b1 = const.tile([128, HT], f32)
            nc.sync.dma_start(out=sb_b1, in_=d_b1[:].rearrange("(t p) -> p t", p=128))
            bv_bc = const.tile([128, DIM], f32)
            nc.gpsimd.dma_start(out=bv_bc, in_=bcast_ap(d_bv))
            bo_bc = const.tile([128, DIM], f32)
            nc.gpsimd.dma_start(out=bo_bc, in_=bcast_ap(d_bo))
            b2_bc = const.tile([128, DIM], f32)
            nc.gpsimd.dma_start(out=b2_bc, in_=bcast_ap(d_b2))
            g1_bc = const.tile([128, DIM], f32)
            nc.gpsimd.dma_start(out=g1_bc, in_=bcast_ap(d_g1))
            bb1_bc = const.tile([128, DIM], f32)
            nc.gpsimd.dma_start(out=bb1_bc, in_=bcast_ap(d_bb1))
            g2_bc = const.tile([128, DIM], f32)
            nc.gpsimd.dma_start(out=g2_bc, in_=bcast_ap(d_g2))
            bb2_bc = const.tile([128, DIM], f32)
            nc.gpsimd.dma_start(out=bb2_bc, in_=bcast_ap(d_bb2))
            ident = const.tile([128, 128], f32)
            make_identity(nc, ident[:])
            ones64 = const.tile([1, 64], f32)
            nc.vector.memset(ones64, 1.0)
            eps_t = const.tile([128, 1], f32)
            nc.vector.memset(eps_t, EPS)

            # ---------- persistent activations ----------
            sb_xblk = big.tile([128, TT, DIM], f32)  # residual x
            sb_l1 = big.tile([128, TT, DIM], f32)

            nc.sync.dma_start(
                out=sb_xblk, in_=d_xb[:].rearrange("(t p) d -> p t d", p=128)
            )

            # attention-scoped residents (freed before the FFN phases)
            attn_res_cm = tc.tile_pool(name="attn_res", bufs=1)
            attn_res = attn_res_cm.__enter__()
            sb_K = attn_res.tile([128, FT, NBLK, BLK], bf16)  # K^T, feat-major
            sb_Q = attn_res.tile([128, FT, BLK], bf16)  # Q^T, feat-major
            sb_V = attn_res.tile([128, ST, HEADS, DK + 1], bf16)  # V + ones col
            sb_zT = attn_res.tile([128, FT, BLK], bf16)  # z^T normalized

            # ============ Phase 1: QKV projections ============
            with (
                tc.tile_pool(name="xw", bufs=1) as xw,
                tc.tile_pool(name="ps1", bufs=4, space="PSUM") as ps1,
                tc.tile_pool(name="ps1v", bufs=4, space="PSUM") as ps1v,
            ):
                if not USE_AG:
                    sb_xT = xw.tile([128, FT, S], bf16)
                    nc.sync.dma_start(
                        out=sb_xT, in_=d_xT[:].rearrange("(t p) n -> p t n", p=128)
                    )
                sb_xTb = xw.tile([128, FT, BLK], bf16)
                nc.sync.dma_start(
                    out=sb_xTb, in_=d_xTb[:].rearrange("(t p) n -> p t n", p=128)
                )
                w_q = xw.tile([128, FT, DIM], bf16)
                nc.sync.dma_start(
                    out=w_q, in_=d_wq[:].rearrange("(t p) o -> p t o", p=128)
                )
                w_k = xw.tile([128, FT, DIM], bf16)
                nc.sync.dma_start(
                    out=w_k, in_=d_wk[:].rearrange("(t p) o -> p t o", p=128)
                )
                w_v = xw.tile([128, FT, DIM], bf16)
                nc.sync.dma_start(
                    out=w_v, in_=d_wv[:].rearrange("(t p) o -> p t o", p=128)
                )

                if USE_AG:
                    # K^T feat-major for the own block only -> bounce -> AG
                    kstage = xw.tile([128, FT, BLK], bf16, tag="kstage")
                    for ft in range(FT):
                        ps = ps1.tile([128, 512], f32, tag="p")
                        for kt in range(FT):
                            nc.tensor.matmul(
                                ps,
                                w_k[:, kt, ft * 128 : (ft + 1) * 128],
                                sb_xTb[:, kt, :],
                                start=(kt == 0),
                                stop=(kt == FT - 1),
                            )
                        nc.vector.tensor_scalar_add(
                            kstage[:, ft, :], ps, sb_bk[:, ft : ft + 1]
                        )
                    nc.sync.dma_start(
                        out=d_kb[:].rearrange("(t p) n -> p t n", p=128), in_=kstage
                    )
                else:
                    # K^T feat-major over the whole batch (replicated)
                    for ft in range(FT):
                        for nt in range(S // 512):
                            ps = ps1.tile([128, 512], f32, tag="p")
                            for kt in range(FT):
                                nc.tensor.matmul(
                                    ps,
                                    w_k[:, kt, ft * 128 : (ft + 1) * 128],
                                    sb_xT[:, kt, nt * 512 : (nt + 1) * 512],
                                    start=(kt == 0),
                                    stop=(kt == FT - 1),
                                )
                            nc.vector.tensor_scalar_add(
                                sb_K[:, ft, nt, :], ps, sb_bk[:, ft : ft + 1]
                            )
                # Q^T feat-major for the core's block
                for ft in range(FT):
                    ps = ps1.tile([128, 512], f32, tag="p")
                    for kt in range(FT):
                        nc.tensor.matmul(
                            ps,
                            w_q[:, kt, ft * 128 : (ft + 1) * 128],
                            sb_xTb[:, kt, :],
                            start=(kt == 0),
                            stop=(kt == FT - 1),
                        )
                    nc.vector.tensor_scalar_add(
                        sb_Q[:, ft, :], ps, sb_bq[:, ft : ft + 1]
                    )
                if USE_AG:
                    # V tok-major for the own block -> bounce -> AG
                    vstage = xw.tile([128, TT, HEADS, DK + 1], bf16, tag="vstage")
                    nc.vector.memset(vstage[:, :, :, DK : DK + 1], 1.0)
                    for tt in range(TT):
                        for nh in range(2):
                            ps = ps1v.tile([128, 384], f32, tag="vp")
                            for kt in range(FT):
                                nc.tensor.matmul(
                                    ps,
                                    sb_xTb[:, kt, tt * 128 : (tt + 1) * 128],
                                    w_v[:, kt, nh * 384 : (nh + 1) * 384],
                                    start=(kt == 0),
                                    stop=(kt == FT - 1),
                                )
                            nc.vector.scalar_tensor_tensor(
                                out=vstage[:, tt, nh * 6 : (nh + 1) * 6, 0:DK],
                                in0=ps[:].rearrange("p (h d) -> p h d", d=DK),
                                scalar=1.0,
                                in1=bv_bc[:, nh * 384 : (nh + 1) * 384].rearrange(
                                    "p (h d) -> p h d", d=DK
                                ),
                                op0=ALU.mult,
                                op1=ALU.add,
                            )
                    nc.sync.dma_start(
                        out=d_vb[:].rearrange("(t p) (h d) -> p t h d", p=128, d=DK + 1),
                        in_=vstage,
                    )
                    # AllGather K and V across the 4-core batch group
                    nc.gpsimd.collective_compute(
                        "AllGather", ALU.bypass, replica_groups=RG,
                        ins=[d_kb[:]], outs=[d_ks[:]],
                    )
                    nc.gpsimd.collective_compute(
                        "AllGather", ALU.bypass, replica_groups=RG,
                        ins=[d_vb[:]], outs=[d_vs[:]],
                    )
                    for b in range(NBLK):
                        nc.sync.dma_start(
                            out=sb_K[:, :, b, :],
                            in_=d_ks[b * DIM : (b + 1) * DIM, :].rearrange(
                                "(t p) n -> p t n", p=128
                            ),
                        )
                    nc.sync.dma_start(
                        out=sb_V,
                        in_=d_vs[:].rearrange(
                            "(t p) (h d) -> p t h d", p=128, d=DK + 1
                        ),
                    )
                else:
                    # V tok-major over the whole batch, laid out [tok, head, dk+1]
                    nc.vector.memset(sb_V[:, :, :, DK : DK + 1], 1.0)
                    for nh in range(2):
                        for tt in range(ST):
                            ps = ps1v.tile([128, 384], f32, tag="vp")
                            for kt in range(FT):
                                nc.tensor.matmul(
                                    ps,
                                    sb_xT[:, kt, tt * 128 : (tt + 1) * 128],
                                    w_v[:, kt, nh * 384 : (nh + 1) * 384],
                                    start=(kt == 0),
                                    stop=(kt == FT - 1),
                                )
                            nc.vector.scalar_tensor_tensor(
                                out=sb_V[:, tt, nh * 6 : (nh + 1) * 6, 0:DK],
                                in0=ps[:].rearrange("p (h d) -> p h d", d=DK),
                                scalar=1.0,
                                in1=bv_bc[:, nh * 384 : (nh + 1) * 384].rearrange(
                                    "p (h d) -> p h d", d=DK
                                ),
                                op0=ALU.mult,
                                op1=ALU.add,
                            )

            if MAX_PHASE >= 2:
                # ============ Phase 2: attention ============
                with (
                    tc.tile_pool(name="expp", bufs=64) as expp,
                    tc.tile_pool(name="attsm", bufs=2) as attsm,
                    tc.tile_pool(name="ps_sc", bufs=4, space="PSUM") as ps_sc,
                    tc.tile_pool(name="ps_z", bufs=1, space="PSUM") as ps_z,
                    tc.tile_pool(name="ps_rb", bufs=2, space="PSUM") as ps_rb,
                ):
                    for hp in range(HEADS // 2):
                        ht = hp
                        # interleave the two heads of a pair kt-by-kt: their
                        # K=64 matmuls sit in disjoint PE row groups (0-63 /
                        # 64-127) so the hardware overlaps adjacent pairs.
                        ets = ([], [])
                        for kt2 in range(ST):
                            for half in (0, 1):
                                ho = half * 64
                                ps = ps_sc.tile([128, BLK], f32, tag="sc")
                                nc.tensor.matmul(
                                    ps,
                                    sb_K[ho : ho + 64, ht, kt2 // 4, (kt2 % 4) * 128 : (kt2 % 4) * 128 + 128],
                                    sb_Q[ho : ho + 64, ht, :],
                                    start=True,
                                    stop=True,
                                )
                                et = expp.tile([128, BLK], bf16, tag="exp")
                                nc.scalar.activation(
                                    et, ps, AF.Exp, scale=sb_msc[:, kt2 : kt2 + 1]
                                )
                                ets[half].append(et)
                        zps = []
                        for half in (0, 1):
                            h = 2 * hp + half
                            zp = ps_z.tile([DK + 1, BLK], f32, tag=f"z{half}")
                            for kt2 in range(ST):
                                nc.tensor.matmul(
                                    zp,
                                    sb_V[:, kt2, h, :],
                                    ets[half][kt2],
                                    start=(kt2 == 0),
                                    stop=(kt2 == ST - 1),
                                )
                            zps.append(zp)
                        for half in (0, 1):
                            ho = half * 64
                            # denominators are huge sums (>= 1): the ~18-bit
                            # fast reciprocal is ~5x cheaper and plenty exact.
                            # (bitwise-trick op: input must be in SBUF, not PSUM)
                            zrow = attsm.tile([1, BLK], f32, tag="zrow")
                            nc.vector.tensor_copy(zrow, zps[half][DK : DK + 1, :])
                            rsum = attsm.tile([1, BLK], f32, tag="rsum")
                            nc.vector.reciprocal_approx_fast(rsum, zrow)
                            rbp = ps_rb.tile([64, BLK], f32, tag="rb")
                            nc.tensor.matmul(
                                rbp, ones64[:], rsum, start=True, stop=True
                            )
                            rb = attsm.tile([64, BLK], f32, tag="rbs")
                            nc.vector.tensor_copy(rb, rbp)
                            nc.vector.tensor_mul(
                                sb_zT[ho : ho + 64, ht, :], zps[half][0:DK, :], rb
                            )

            if MAX_PHASE >= 3:
                # ============ Phase 3: O proj + LN1 (+residual) ============
                def layer_norm_to(out_ap, x_ap, g_bc_t, resid_ap, pool):
                    s = pool.tile([128, 1], f32, tag="ln_s")
                    nc.vector.tensor_reduce(s, x_ap, axis=AX.X, op=ALU.add)
                    mean = pool.tile([128, 1], f32, tag="ln_m")
                    nc.scalar.mul(mean, s, 1.0 / DIM)
                    xc = pool.tile([128, DIM], f32, tag="ln_xc")
                    nc.vector.tensor_scalar(xc, x_ap, mean, None, op0=ALU.subtract)
                    junk = pool.tile([128, DIM], f32, tag="ln_j")
                    var = pool.tile([128, 1], f32, tag="ln_v")
                    # (tensor_tensor_reduce crashes the device on this runtime;
                    # scalar_tensor_tensor with accum_out works)
                    nc.vector.scalar_tensor_tensor(
                        out=junk, in0=xc, scalar=1.0, in1=xc,
                        op0=ALU.mult, op1=ALU.mult, accum_out=var,
                    )
                    nc.vector.tensor_scalar_mul(var, var, 1.0 / DIM)
                    sd = pool.tile([128, 1], f32, tag="ln_sd")
                    nc.scalar.activation(sd, var, AF.Sqrt, bias=eps_t[:])
                    rstd = pool.tile([128, 1], f32, tag="ln_r")
                    nc.vector.reciprocal(rstd, sd)
                    t = pool.tile([128, DIM], f32, tag="ln_t")
                    nc.vector.tensor_scalar(t, xc, rstd, None, op0=ALU.mult)
                    tg = pool.tile([128, DIM], f32, tag="ln_tg")
                    nc.vector.tensor_mul(tg, t, g_bc_t)
                    nc.vector.tensor_add(out_ap, tg, resid_ap)

                with (
                    tc.tile_pool(name="wo_p", bufs=1) as wo_p,
                    tc.tile_pool(name="ln1p", bufs=2) as ln1p,
                    tc.tile_pool(name="ps_o", bufs=4, space="PSUM") as ps_o,
                ):
                    w_o = wo_p.tile([128, FT, DIM], bf16)
                    nc.sync.dma_start(
                        out=w_o, in_=d_wo[:].rearrange("(t p) o -> p t o", p=128)
                    )
                    for tt in range(TT):
                        l1pre = ln1p.tile([128, DIM], f32, tag="l1pre")
                        for nh in range(2):
                            ps = ps_o.tile([128, 384], f32, tag="op")
                            for kt in range(FT):
                                nc.tensor.matmul(
                                    ps,
                                    sb_zT[:, kt, tt * 128 : (tt + 1) * 128],
                                    w_o[:, kt, nh * 384 : (nh + 1) * 384],
                                    start=(kt == 0),
                                    stop=(kt == FT - 1),
                                )
                            nc.vector.scalar_tensor_tensor(
                                out=l1pre[:, nh * 384 : (nh + 1) * 384],
                                in0=ps,
                                scalar=1.0,
                                in1=bo_bc[:, nh * 384 : (nh + 1) * 384],
                                op0=ALU.mult,
                                op1=ALU.add,
                            )
                        xb1 = ln1p.tile([128, DIM], f32, tag="xb1")
                        nc.vector.tensor_add(xb1, sb_xblk[:, tt, :], bb1_bc)
                        layer_norm_to(sb_l1[:, tt, :], l1pre[:], g1_bc, xb1, ln1p)

            attn_res_cm.__exit__(None, None, None)
            sb_hT = big.tile([128, HT, BLK], bf16)  # relu(ffn1)^T, hid-major

            if MAX_PHASE >= 4:
                # ============ Phase 4: transpose l1, FFN1 ============
                with (
                    tc.tile_pool(name="w1_p", bufs=1) as w1_p,
                    tc.tile_pool(name="l1t_p", bufs=1) as l1t_p,
                    tc.tile_pool(name="ps_t", bufs=2, space="PSUM") as ps_t,
                    tc.tile_pool(name="ps_f1", bufs=4, space="PSUM") as ps_f1,
                ):
                    w1_t = []
                    for kt in range(FT):
                        wt = w1_p.tile([128, HID], bf16, tag=f"w1_{kt}")
                        nc.sync.dma_start(
                            out=wt, in_=d_w1[kt * 128 : (kt + 1) * 128, :]
                        )
                        w1_t.append(wt)
                    sb_l1T = l1t_p.tile([128, FT, BLK], bf16)
                    for ft in range(FT):
                        for tt in range(TT):
                            pst = ps_t.tile([128, 128], f32, tag="tp")
                            nc.tensor.transpose(
                                pst, sb_l1[:, tt, ft * 128 : (ft + 1) * 128], ident[:]
                            )
                            nc.scalar.copy(
                                sb_l1T[:, ft, tt * 128 : (tt + 1) * 128], pst
                            )
                    for ht2 in range(HT):
                        ps = ps_f1.tile([128, BLK], f32, tag="f1")
                        for kt in range(FT):
                            nc.tensor.matmul(
                                ps,
                                w1_t[kt][:, ht2 * 128 : (ht2 + 1) * 128],
                                sb_l1T[:, kt, :],
                                start=(kt == 0),
                                stop=(kt == FT - 1),
                            )
                        # relu(x + b1) on DVE: (x add b1) max 0
                        nc.vector.tensor_scalar(
                            sb_hT[:, ht2, :], ps, sb_b1[:, ht2 : ht2 + 1], 0.0,
                            op0=ALU.add, op1=ALU.max,
                        )

            if MAX_PHASE >= 5:
                # ============ Phase 5: FFN2 + LN2 + out ============
                with (
                    tc.tile_pool(name="w2_p", bufs=1) as w2_p,
                    tc.tile_pool(name="ln2p", bufs=2) as ln2p,
                    tc.tile_pool(name="outp", bufs=3) as outp,
                    tc.tile_pool(name="ps_f2", bufs=4, space="PSUM") as ps_f2,
                ):
                    w2_t = []
                    for kt in range(HT):
                        wt = w2_p.tile([128, DIM], bf16, tag=f"w2_{kt}")
                        nc.sync.dma_start(
                            out=wt, in_=d_w2[kt * 128 : (kt + 1) * 128, :]
                        )
                        w2_t.append(wt)
                    out_r = d_out[:].rearrange("(t p) d -> p t d", p=128)
                    for tt in range(TT):
                        f2pre = ln2p.tile([128, DIM], f32, tag="f2pre")
                        for nh in range(2):
                            ps = ps_f2.tile([128, 384], f32, tag="f2")
                            for kt in range(HT):
                                nc.tensor.matmul(
                                    ps,
                                    sb_hT[:, kt, tt * 128 : (tt + 1) * 128],
                                    w2_t[kt][:, nh * 384 : (nh + 1) * 384],
                                    start=(kt == 0),
                                    stop=(kt == HT - 1),
                                )
                            nc.vector.scalar_tensor_tensor(
                                out=f2pre[:, nh * 384 : (nh + 1) * 384],
                                in0=ps,
                                scalar=1.0,
                                in1=b2_bc[:, nh * 384 : (nh + 1) * 384],
                                op0=ALU.mult,
                                op1=ALU.add,
                            )
                        l1b = ln2p.tile([128, DIM], f32, tag="l1b")
                        nc.vector.tensor_add(l1b, sb_l1[:, tt, :], bb2_bc)
                        o_sb = outp.tile([128, DIM], f32, tag="osb")
                        layer_norm_to(o_sb[:], f2pre[:], g2_bc, l1b, ln2p)
                        nc.sync.dma_start(out=out_r[:, tt, :], in_=o_sb)

    return nc


def _get_nc(finalized=True):
    if "nc" not in _CACHE:
        _CACHE["nc"] = _build_program()
    nc = _CACHE["nc"]
    if finalized and not nc.is_finalized():
        nc.finalize()
    return nc


def make_in_maps(inputs: dict) -> list:
    x = np.asarray(inputs["x_n"], np.float32).reshape(B, S, DIM)
    mask = np.asarray(inputs["mask"]).reshape(B, S)
    w = {
        k: np.ascontiguousarray(np.asarray(inputs[k], np.float32).astype(BF16))
        for k in ("wq", "wk", "wv", "wo", "w1", "w2")
    }
    vecs = {
        "bq": inputs["bq"], "bk": inputs["bk"], "bv": inputs["bv"],
        "bo": inputs["bo"], "b1": inputs["b1"], "b2": inputs["b2"],
        "g1": inputs["ln1_g"], "bb1": inputs["ln1_b"],
        "g2": inputs["ln2_g"], "bb2": inputs["ln2_b"],
    }
    vecs = {k: np.ascontiguousarray(np.asarray(v, np.float32)) for k, v in vecs.items()}
    in_maps = []
    for c in range(N_CORES):
        b, blk = c // NBLK, c % NBLK
        xb = x[b]
        xT = None if USE_AG else np.ascontiguousarray(xb.T.astype(BF16))
        xblk = np.ascontiguousarray(xb[blk * BLK : (blk + 1) * BLK])
        xTb = np.ascontiguousarray(xblk.T.astype(BF16))
        msc = (mask[b].astype(np.float32) != 0).astype(np.float32) * ISCALE
        m = {"xTb": xTb, "xb": xblk, "msc": msc}
        if not USE_AG:
            m["xT"] = xT
        m.update(w)
        m.update(vecs)
        in_maps.append(m)
    return in_maps


def assemble(per_core_out: list) -> np.ndarray:
    blocks = [np.asarray(o, np.float32) for o in per_core_out]
    full = np.concatenate(blocks, axis=0).reshape(B, S, DIM)
    return full


def kernel(**inputs) -> np.ndarray:
    from concourse.bass_utils import run_bass_kernel_spmd

    nc = _get_nc()
    in_maps = make_in_maps(inputs)
    res = run_bass_kernel_spmd(nc, in_maps, list(range(N_CORES)))
    return assemble([r["out"] for r in res.results])



# revision 14
# speedup vs baseline: 1.4730x; 1.2287x over previous
"""Trainium2 Bass kernel for a dense transformer encoder layer (v2).

Model (faithful to the oracle):
  q,k,v = x@wq+bq, x@wk+bk, x@wv+bv          (12 heads, dk=64, DIM=768)
  scores = q@k^T / sqrt(768)  (note: sqrt(dim_model), not sqrt(dk))
  scores[mask==0] = 1e-11  (NOT -inf; masked keys still contribute ~1/Z)
  attn = softmax(scores); z = attn@v; o = z@wo+bo
  l1 = x + LN(o);  ffn = relu(l1@w1+b1)@w2+b2;  out = l1 + LN(ffn)

Sharding: 4096 tokens (B=2,S=2048) split 8 ways -> 512 tokens/core.
Cores 0-3 own batch 0, cores 4-7 batch 1. No collectives: each core
computes K/V itself — but only for the UNMASKED keys of its batch.

Mask compaction: masked keys (~half) all get score 1e-11, i.e. exp==1
(fp32), for every query/head. Their attention contribution is a
query-independent constant: C_h = sum_masked v_k (numerator) and
m = #masked (denominator). The host compacts unmasked tokens into a
padded [KC] buffer (KC=1280 >> max plausible count), computes the tiny
C correction in numpy, and the device runs attention only over the
compacted keys, seeding the attn@v PSUM accumulation with C via a K=1
matmul. Padded key slots have K=V=0 exactly (biases are added inside
the matmuls via a 0/1 validity row), so exp(score)=1 * V=0 contributes
nothing; the validity row also zeroes their denominator entry.

Softmax: scores are built k-major (scoresT [kpos, q]); all compacted
keys are unmasked so exp(ISCALE * s) uses a constant scale. The
denominator comes from a ones column in V (1 for real keys, 0 for
pads, via one replicated DMA); normalization happens after attn@v via
a rank-1 matmul broadcast of the fast reciprocal (input staged to SBUF
— the bit-trick op misreads PSUM).
"""

import math
import os
import sys

import numpy as np

for _p in ("/opt/trn_rl_repo", os.path.expanduser("~/.axon_site/_ro/trn_rl_repo")):
    if os.path.isdir(_p) and _p not in sys.path:
        sys.path.insert(0, _p)

import ml_dtypes  # noqa: E402

BF16 = ml_dtypes.bfloat16

DIM = 768
HEADS = 12
DK = 64
HID = 4 * DIM  # 3072
B, S = 2, 2048
N_CORES = 8
BLK = 512            # tokens per core
NBLK = S // BLK      # 4 blocks per batch
KC = 1280            # compacted-key capacity (10 tiles; ~11 sigma above E[n_u])
EPS = 1e-5
ISCALE = 1.0 / math.sqrt(DIM)

FT = DIM // 128   # 6 feature tiles
TT = BLK // 128   # 4 token tiles per core block
STC = KC // 128   # 10 compacted key tiles
HT = HID // 128   # 24 hidden tiles

_CACHE: dict = {}
MAX_PHASE = int(os.environ.get("BASS_KERNEL_PHASES", "5"))


def _build_program():
    import concourse.bass as bass
    import concourse.mybir as mybir
    import concourse.tile as tile
    from concourse import bacc
    from concourse.masks import make_identity

    f32 = mybir.dt.float32
    bf16 = mybir.dt.bfloat16
    AF = mybir.ActivationFunctionType
    ALU = mybir.AluOpType
    AX = mybir.AxisListType

    nc = bacc.Bacc()

    # ---- per-core DRAM I/O ----
    d_xTb = nc.dram_tensor("xTb", [DIM, BLK], bf16, kind="ExternalInput")
    d_xb = nc.dram_tensor("xb", [BLK, DIM], f32, kind="ExternalInput")
    d_xkvT = nc.dram_tensor("xkvT", [DIM, KC], bf16, kind="ExternalInput")
    d_onesc = nc.dram_tensor("onesc", [KC], bf16, kind="ExternalInput")
    d_onescv = nc.dram_tensor("onescv", [STC, HEADS, 128], bf16, kind="ExternalInput")
    d_wq = nc.dram_tensor("wq", [DIM, DIM], bf16, kind="ExternalInput")
    d_wk = nc.dram_tensor("wk", [DIM, DIM], bf16, kind="ExternalInput")
    d_wv = nc.dram_tensor("wv", [DIM, DIM], bf16, kind="ExternalInput")
    d_wo = nc.dram_tensor("wo", [DIM, DIM], bf16, kind="ExternalInput")
    d_w1 = nc.dram_tensor("w1", [DIM, HID], bf16, kind="ExternalInput")
    d_w2 = nc.dram_tensor("w2", [HID, DIM], bf16, kind="ExternalInput")
    d_bkrow = nc.dram_tensor("bkrow", [DIM], bf16, kind="ExternalInput")
    d_bvrow = nc.dram_tensor("bvrow", [DIM], bf16, kind="ExternalInput")
    d_bqrow = nc.dram_tensor("bqrow", [DIM], bf16, kind="ExternalInput")
    d_crow = nc.dram_tensor("crow", [HEADS * (DK + 1)], f32, kind="ExternalInput")
    d_bo = nc.dram_tensor("bo", [DIM], f32, kind="ExternalInput")
    d_b1 = nc.dram_tensor("b1", [HID], f32, kind="ExternalInput")
    d_b2 = nc.dram_tensor("b2", [DIM], f32, kind="ExternalInput")
    d_g1 = nc.dram_tensor("g1", [DIM], f32, kind="ExternalInput")
    d_bb1 = nc.dram_tensor("bb1", [DIM], f32, kind="ExternalInput")
    d_g2 = nc.dram_tensor("g2", [DIM], f32, kind="ExternalInput")
    d_bb2 = nc.dram_tensor("bb2", [DIM], f32, kind="ExternalInput")
    d_out = nc.dram_tensor("out", [BLK, DIM], f32, kind="ExternalOutput")

    KCH = [(0, 512), (512, 1024), (1024, KC)]  # K-proj N chunks

    def bcast_ap(handle, n=128):
        ap = handle[:]
        return bass.AP(tensor=ap.tensor, offset=ap.offset, ap=[[0, n]] + list(ap.ap))

    with tile.TileContext(nc) as tc:
        with (
            tc.tile_pool(name="const", bufs=1) as const,
            tc.tile_pool(name="bigres", bufs=1) as big,
        ):
            # ---------- constants ----------
            sb_b1 = const.tile([128, HT], f32)
            nc.sync.dma_start(out=sb_b1, in_=d_b1[:].rearrange("(t p) -> p t", p=128))
            bo_bc = const.tile([128, DIM], f32)
            nc.gpsimd.dma_start(out=bo_bc, in_=bcast_ap(d_bo))
            b2_bc = const.tile([128, DIM], f32)
            nc.gpsimd.dma_start(out=b2_bc, in_=bcast_ap(d_b2))
            g1_bc = const.tile([128, DIM], f32)
            nc.gpsimd.dma_start(out=g1_bc, in_=bcast_ap(d_g1))
            bb1_bc = const.tile([128, DIM], f32)
            nc.gpsimd.dma_start(out=bb1_bc, in_=bcast_ap(d_bb1))
            g2_bc = const.tile([128, DIM], f32)
            nc.gpsimd.dma_start(out=g2_bc, in_=bcast_ap(d_g2))
            bb2_bc = const.tile([128, DIM], f32)
            nc.gpsimd.dma_start(out=bb2_bc, in_=bcast_ap(d_bb2))
            ident = const.tile([128, 128], f32)
            make_identity(nc, ident[:])
            ones64 = const.tile([1, 64], f32)
            nc.vector.memset(ones64, 1.0)
            ones512f = const.tile([1, BLK], f32)
            nc.vector.memset(ones512f, 1.0)
            ones512b = const.tile([1, BLK], bf16)
            nc.vector.memset(ones512b, 1.0)
            eps_t = const.tile([128, 1], f32)
            nc.vector.memset(eps_t, EPS)
            sb_bkrow = const.tile([1, DIM], bf16)
            nc.sync.dma_start(out=sb_bkrow, in_=bcast_ap(d_bkrow, 1))
            sb_bvrow = const.tile([1, DIM], bf16)
            nc.sync.dma_start(out=sb_bvrow, in_=bcast_ap(d_bvrow, 1))
            sb_bqrow = const.tile([1, DIM], bf16)
            nc.sync.dma_start(out=sb_bqrow, in_=bcast_ap(d_bqrow, 1))
            sb_crow = const.tile([1, HEADS * (DK + 1)], f32)
            nc.sync.dma_start(out=sb_crow, in_=bcast_ap(d_crow, 1))
            sb_onesc = const.tile([1, KC], bf16)
            nc.sync.dma_start(out=sb_onesc, in_=bcast_ap(d_onesc, 1))

            # ---------- persistent activations ----------
            sb_xblk = big.tile([128, TT, DIM], f32)  # residual x
            sb_l1 = big.tile([128, TT, DIM], f32)

            nc.sync.dma_start(
                out=sb_xblk, in_=d_xb[:].rearrange("(t p) d -> p t d", p=128)
            )

            # attention-scoped residents (freed before the FFN phases)
            attn_res_cm = tc.tile_pool(name="attn_res", bufs=1)
            attn_res = attn_res_cm.__enter__()
            sb_K = attn_res.tile([128, FT, KC], bf16)  # K^T, feat-major
            sb_Q = attn_res.tile([128, FT, BLK], bf16)  # Q^T, feat-major
            sb_V = attn_res.tile([128, STC, HEADS, DK + 1], bf16)  # V + ones col
            sb_zT = attn_res.tile([128, FT, BLK], bf16)  # z^T normalized

            # ones column of V: validity row, host-replicated across heads
            nc.gpsimd.dma_start(
                out=sb_V[:, :, :, DK : DK + 1],
                in_=d_onescv[:].rearrange("t h p -> p t h"),
            )

            # ============ Phase 1: QKV projections ============
            with (
                tc.tile_pool(name="xw", bufs=1) as xw,
                tc.tile_pool(name="ps1", bufs=4, space="PSUM") as ps1,
                tc.tile_pool(name="ps1v", bufs=4, space="PSUM") as ps1v,
            ):
                sb_xkvT = xw.tile([128, FT, KC], bf16)
                nc.sync.dma_start(
                    out=sb_xkvT, in_=d_xkvT[:].rearrange("(t p) n -> p t n", p=128)
                )
                sb_xTb = xw.tile([128, FT, BLK], bf16)
                nc.sync.dma_start(
                    out=sb_xTb, in_=d_xTb[:].rearrange("(t p) n -> p t n", p=128)
                )
                w_k = xw.tile([128, FT, DIM], bf16)
                nc.sync.dma_start(
                    out=w_k, in_=d_wk[:].rearrange("(t p) o -> p t o", p=128)
                )
                w_q = xw.tile([128, FT, DIM], bf16)
                nc.sync.dma_start(
                    out=w_q, in_=d_wq[:].rearrange("(t p) o -> p t o", p=128)
                )
                w_v = xw.tile([128, FT, DIM], bf16)
                nc.sync.dma_start(
                    out=w_v, in_=d_wv[:].rearrange("(t p) o -> p t o", p=128)
                )

                # K^T feat-major over compacted keys; bias via validity row
                for ft in range(FT):
                    for c0, c1 in KCH:
                        ps = ps1.tile([128, c1 - c0], f32, tag="p")
                        for kt in range(FT):
                            nc.tensor.matmul(
                                ps,
                                w_k[:, kt, ft * 128 : (ft + 1) * 128],
                                sb_xkvT[:, kt, c0:c1],
                                start=(kt == 0),
                                stop=False,
                            )
                        nc.tensor.matmul(
                            ps,
                            sb_bkrow[0:1, ft * 128 : (ft + 1) * 128],
                            sb_onesc[0:1, c0:c1],
                            start=False,
                            stop=True,
                        )
                        nc.scalar.copy(sb_K[:, ft, c0:c1], ps)
                # Q^T feat-major for the core's block
                for ft in range(FT):
                    ps = ps1.tile([128, BLK], f32, tag="p")
                    for kt in range(FT):
                        nc.tensor.matmul(
                            ps,
                            w_q[:, kt, ft * 128 : (ft + 1) * 128],
                            sb_xTb[:, kt, :],
                            start=(kt == 0),
                            stop=False,
                        )
                    nc.tensor.matmul(
                        ps,
                        sb_bqrow[0:1, ft * 128 : (ft + 1) * 128],
                        ones512b[:],
                        start=False,
                        stop=True,
                    )
                    nc.scalar.copy(sb_Q[:, ft, :], ps)
                # V tok-major over compacted keys, [tok, head, dk]
                for nh in range(2):
                    for tt in range(STC):
                        ps = ps1v.tile([128, 384], f32, tag="vp")
                        for kt in range(FT):
                            nc.tensor.matmul(
                                ps,
                                sb_xkvT[:, kt, tt * 128 : (tt + 1) * 128],
                                w_v[:, kt, nh * 384 : (nh + 1) * 384],
                                start=(kt == 0),
                                stop=False,
                            )
                        nc.tensor.matmul(
                            ps,
                            sb_onesc[0:1, tt * 128 : (tt + 1) * 128],
                            sb_bvrow[0:1, nh * 384 : (nh + 1) * 384],
                            start=False,
                            stop=True,
                        )
                        nc.vector.tensor_copy(
                            sb_V[:, tt, nh * 6 : (nh + 1) * 6, 0:DK],
                            ps[:].rearrange("p (h d) -> p h d", d=DK),
                        )

            if MAX_PHASE >= 2:
                # ============ Phase 2: attention ============
                with (
                    tc.tile_pool(name="expp", bufs=40) as expp,
                    tc.tile_pool(name="attsm", bufs=2) as attsm,
                    tc.tile_pool(name="ps_sc", bufs=4, space="PSUM") as ps_sc,
                    tc.tile_pool(name="ps_z", bufs=1, space="PSUM") as ps_z,
                    tc.tile_pool(name="ps_rb", bufs=2, space="PSUM") as ps_rb,
                ):
                    for hp in range(HEADS // 2):
                        ht = hp
                        # interleave the two heads of a pair kt-by-kt: their
                        # K=64 matmuls sit in disjoint PE row groups (0-63 /
                        # 64-127) so the hardware overlaps adjacent pairs.
                        ets = ([], [])
                        for kt2 in range(STC):
                            for half in (0, 1):
                                ho = half * 64
                                ps = ps_sc.tile([128, BLK], f32, tag="sc")
                                nc.tensor.matmul(
                                    ps,
                                    sb_K[ho : ho + 64, ht, kt2 * 128 : (kt2 + 1) * 128],
                                    sb_Q[ho : ho + 64, ht, :],
                                    start=True,
                                    stop=True,
                                )
                                et = expp.tile([128, BLK], bf16, tag="exp")
                                nc.scalar.activation(et, ps, AF.Exp, scale=ISCALE)
                                ets[half].append(et)
                        zps = []
                        for half in (0, 1):
                            h = 2 * hp + half
                            zp = ps_z.tile([DK + 1, BLK], f32, tag=f"z{half}")
                            # seed with the masked-keys correction row
                            nc.tensor.matmul(
                                zp,
                                sb_crow[0:1, h * (DK + 1) : (h + 1) * (DK + 1)],
                                ones512f[:],
                                start=True,
                                stop=False,
                            )
                            for kt2 in range(STC):
                                nc.tensor.matmul(
                                    zp,
                                    sb_V[:, kt2, h, :],
                                    ets[half][kt2],
                                    start=False,
                                    stop=(kt2 == STC - 1),
                                )
                            zps.append(zp)
                        for half in (0, 1):
                            ho = half * 64
                            # denominators are huge sums (>= 1): the ~18-bit
                            # fast reciprocal is ~5x cheaper and plenty exact.
                            # (bitwise-trick op: input must be in SBUF, not PSUM)
                            zrow = attsm.tile([1, BLK], f32, tag="zrow")
                            nc.vector.tensor_copy(zrow, zps[half][DK : DK + 1, :])
                            rsum = attsm.tile([1, BLK], f32, tag="rsum")
                            nc.vector.reciprocal_approx_fast(rsum, zrow)
                            rbp = ps_rb.tile([64, BLK], f32, tag="rb")
                            nc.tensor.matmul(
                                rbp, ones64[:], rsum, start=True, stop=True
                            )
                            rb = attsm.tile([64, BLK], f32, tag="rbs")
                            nc.vector.tensor_copy(rb, rbp)
                            nc.vector.tensor_mul(
                                sb_zT[ho : ho + 64, ht, :], zps[half][0:DK, :], rb
                            )

            if MAX_PHASE >= 3:
                # ============ Phase 3: O proj + LN1 (+residual) ============
                def layer_norm_to(out_ap, x_ap, g_bc_t, resid_ap, pool):
                    s = pool.tile([128, 1], f32, tag="ln_s")
                    nc.vector.tensor_reduce(s, x_ap, axis=AX.X, op=ALU.add)
                    junk = pool.tile([128, DIM], f32, tag="ln_j")
                    ssq = pool.tile([128, 1], f32, tag="ln_q")
                    # (tensor_tensor_reduce crashes the device on this runtime;
                    # scalar_tensor_tensor with accum_out works)
                    nc.vector.scalar_tensor_tensor(
                        out=junk, in0=x_ap, scalar=1.0, in1=x_ap,
                        op0=ALU.mult, op1=ALU.mult, accum_out=ssq,
                    )
                    negmean = pool.tile([128, 1], f32, tag="ln_m")
                    nc.scalar.mul(negmean, s, -1.0 / DIM)
                    # var = E[x^2] - mean^2
                    m2 = pool.tile([128, 1], f32, tag="ln_m2")
                    nc.vector.tensor_mul(m2, negmean, negmean)
                    var = pool.tile([128, 1], f32, tag="ln_v")
                    nc.vector.scalar_tensor_tensor(
                        out=var, in0=ssq, scalar=1.0 / DIM, in1=m2,
                        op0=ALU.mult, op1=ALU.subtract,
                    )
                    sd = pool.tile([128, 1], f32, tag="ln_sd")
                    nc.scalar.activation(sd, var, AF.Sqrt, bias=eps_t[:])
                    rstd = pool.tile([128, 1], f32, tag="ln_r")
                    nc.vector.reciprocal(rstd, sd)
                    nmr = pool.tile([128, 1], f32, tag="ln_nm")
                    nc.vector.tensor_mul(nmr, negmean, rstd)
                    # x*rstd on ACT; then (x*rstd - mu*rstd) * g fused on DVE
                    nrm = pool.tile([128, DIM], f32, tag="ln_t")
                    nc.scalar.mul(nrm, x_ap, rstd[:])
                    tg = pool.tile([128, DIM], f32, tag="ln_tg")
                    nc.vector.scalar_tensor_tensor(
                        out=tg, in0=nrm, scalar=nmr[:], in1=g_bc_t,
                        op0=ALU.add, op1=ALU.mult,
                    )
                    nc.vector.tensor_add(out_ap, tg, resid_ap)

                with (
                    tc.tile_pool(name="wo_p", bufs=1) as wo_p,
                    tc.tile_pool(name="ln1p", bufs=2) as ln1p,
                    tc.tile_pool(name="ps_o", bufs=4, space="PSUM") as ps_o,
                ):
                    w_o = wo_p.tile([128, FT, DIM], bf16)
                    nc.sync.dma_start(
                        out=w_o, in_=d_wo[:].rearrange("(t p) o -> p t o", p=128)
                    )
                    for tt in range(TT):
                        l1pre = ln1p.tile([128, DIM], f32, tag="l1pre")
                        for nh in range(2):
                            ps = ps_o.tile([128, 384], f32, tag="op")
                            for kt in range(FT):
                                nc.tensor.matmul(
                                    ps,
                                    sb_zT[:, kt, tt * 128 : (tt + 1) * 128],
                                    w_o[:, kt, nh * 384 : (nh + 1) * 384],
                                    start=(kt == 0),
                                    stop=(kt == FT - 1),
                                )
                            nc.vector.scalar_tensor_tensor(
                                out=l1pre[:, nh * 384 : (nh + 1) * 384],
                                in0=ps,
                                scalar=1.0,
                                in1=bo_bc[:, nh * 384 : (nh + 1) * 384],
                                op0=ALU.mult,
                                op1=ALU.add,
                            )
                        xb1 = ln1p.tile([128, DIM], f32, tag="xb1")
                        nc.vector.tensor_add(xb1, sb_xblk[:, tt, :], bb1_bc)
                        layer_norm_to(sb_l1[:, tt, :], l1pre[:], g1_bc, xb1, ln1p)

            attn_res_cm.__exit__(None, None, None)
            sb_hT = big.tile([128, HT, BLK], bf16)  # relu(ffn1)^T, hid-major

            if MAX_PHASE >= 4:
                # ============ Phase 4: transpose l1, FFN1 ============
                with (
                    tc.tile_pool(name="w1_p", bufs=1) as w1_p,
                    tc.tile_pool(name="l1t_p", bufs=1) as l1t_p,
                    tc.tile_pool(name="ps_t", bufs=2, space="PSUM") as ps_t,
                    tc.tile_pool(name="ps_f1", bufs=4, space="PSUM") as ps_f1,
                ):
                    w1_t = []
                    for kt in range(FT):
                        wt = w1_p.tile([128, HID], bf16, tag=f"w1_{kt}")
                        nc.sync.dma_start(
                            out=wt, in_=d_w1[kt * 128 : (kt + 1) * 128, :]
                        )
                        w1_t.append(wt)
                    sb_l1T = l1t_p.tile([128, FT, BLK], bf16)
                    for ft in range(FT):
                        for tt in range(TT):
                            pst = ps_t.tile([128, 128], f32, tag="tp")
                            nc.tensor.transpose(
                                pst, sb_l1[:, tt, ft * 128 : (ft + 1) * 128], ident[:]
                            )
                            nc.scalar.copy(
                                sb_l1T[:, ft, tt * 128 : (tt + 1) * 128], pst
                            )
                    for ht2 in range(HT):
                        ps = ps_f1.tile([128, BLK], f32, tag="f1")
                        for kt in range(FT):
                            nc.tensor.matmul(
                                ps,
                                w1_t[kt][:, ht2 * 128 : (ht2 + 1) * 128],
                                sb_l1T[:, kt, :],
                                start=(kt == 0),
                                stop=(kt == FT - 1),
                            )
                        # relu(x + b1) on DVE: (x add b1) max 0
                        nc.vector.tensor_scalar(
                            sb_hT[:, ht2, :], ps, sb_b1[:, ht2 : ht2 + 1], 0.0,
                            op0=ALU.add, op1=ALU.max,
                        )

            if MAX_PHASE >= 5:
                # ============ Phase 5: FFN2 + LN2 + out ============
                with (
                    tc.tile_pool(name="w2_p", bufs=1) as w2_p,
                    tc.tile_pool(name="ln2p", bufs=2) as ln2p,
                    tc.tile_pool(name="outp", bufs=3) as outp,
                    tc.tile_pool(name="ps_f2", bufs=4, space="PSUM") as ps_f2,
                ):
                    w2_t = []
                    for kt in range(HT):
                        wt = w2_p.tile([128, DIM], bf16, tag=f"w2_{kt}")
                        nc.sync.dma_start(
                            out=wt, in_=d_w2[kt * 128 : (kt + 1) * 128, :]
                        )
                        w2_t.append(wt)
                    out_r = d_out[:].rearrange("(t p) d -> p t d", p=128)
                    for tt in range(TT):
                        f2pre = ln2p.tile([128, DIM], f32, tag="f2pre")
                        for nh in range(2):
                            ps = ps_f2.tile([128, 384], f32, tag="f2")
                            for kt in range(HT):
                                nc.tensor.matmul(
                                    ps,
                                    sb_hT[:, kt, tt * 128 : (tt + 1) * 128],
                                    w2_t[kt][:, nh * 384 : (nh + 1) * 384],
                                    start=(kt == 0),
                                    stop=(kt == HT - 1),
                                )
                            nc.vector.scalar_tensor_tensor(
                                out=f2pre[:, nh * 384 : (nh + 1) * 384],
                                in0=ps,
                                scalar=1.0,
                                in1=b2_bc[:, nh * 384 : (nh + 1) * 384],
                                op0=ALU.mult,
                                op1=ALU.add,
                            )
                        l1b = ln2p.tile([128, DIM], f32, tag="l1b")
                        nc.vector.tensor_add(l1b, sb_l1[:, tt, :], bb2_bc)
                        o_sb = outp.tile([128, DIM], f32, tag="osb")
                        layer_norm_to(o_sb[:], f2pre[:], g2_bc, l1b, ln2p)
                        nc.sync.dma_start(out=out_r[:, tt, :], in_=o_sb)

    return nc


def _get_nc(finalized=True):
    if "nc" not in _CACHE:
        _CACHE["nc"] = _build_program()
    nc = _CACHE["nc"]
    if finalized and not nc.is_finalized():
        nc.finalize()
    return nc


def make_in_maps(inputs: dict) -> list:
    x = np.asarray(inputs["x_n"], np.float32).reshape(B, S, DIM)
    mask = np.asarray(inputs["mask"]).reshape(B, S)
    w = {
        k: np.ascontiguousarray(np.asarray(inputs[k], np.float32).astype(BF16))
        for k in ("wq", "wk", "wv", "wo", "w1", "w2")
    }
    vecs = {
        "bo": inputs["bo"], "b1": inputs["b1"], "b2": inputs["b2"],
        "g1": inputs["ln1_g"], "bb1": inputs["ln1_b"],
        "g2": inputs["ln2_g"], "bb2": inputs["ln2_b"],
    }
    vecs = {k: np.ascontiguousarray(np.asarray(v, np.float32)) for k, v in vecs.items()}
    brows = {
        "bkrow": np.asarray(inputs["bk"], np.float32).astype(BF16),
        "bvrow": np.asarray(inputs["bv"], np.float32).astype(BF16),
        "bqrow": np.asarray(inputs["bq"], np.float32).astype(BF16),
    }

    # per-batch compaction + masked-keys correction
    per_batch = []
    for b in range(B):
        mb = mask[b] != 0
        idx = np.nonzero(mb)[0]
        n_u = len(idx)
        if n_u > KC:
            raise RuntimeError(
                f"unmasked key count {n_u} exceeds compiled capacity {KC}"
            )
        xkv = np.zeros((KC, DIM), np.float32)
        xkv[:n_u] = x[b][idx]
        xkvT = np.ascontiguousarray(xkv.T.astype(BF16))
        onesc = np.zeros(KC, np.float32)
        onesc[:n_u] = 1.0
        msum = x[b][~mb].astype(np.float64).sum(axis=0)
        mcount = float((~mb).sum())
        wv64 = np.asarray(inputs["wv"], np.float64)
        bv64 = np.asarray(inputs["bv"], np.float64)
        cvec = (msum @ wv64 + mcount * bv64).astype(np.float32)  # [DIM]
        crow = np.zeros(HEADS * (DK + 1), np.float32)
        ch = cvec.reshape(HEADS, DK)
        for h in range(HEADS):
            crow[h * (DK + 1) : h * (DK + 1) + DK] = ch[h]
            crow[h * (DK + 1) + DK] = mcount
        onesc_bf = onesc.astype(BF16)
        onescv = np.ascontiguousarray(
            np.broadcast_to(
                onesc_bf.reshape(STC, 1, 128), (STC, HEADS, 128)
            )
        )
        per_batch.append(
            {"xkvT": xkvT, "onesc": onesc_bf, "onescv": onescv, "crow": crow}
        )

    in_maps = []
    for c in range(N_CORES):
        b, blk = c // NBLK, c % NBLK
        xb = x[b]
        xblk = np.ascontiguousarray(xb[blk * BLK : (blk + 1) * BLK])
        xTb = np.ascontiguousarray(xblk.T.astype(BF16))
        m = {"xTb": xTb, "xb": xblk}
        m.update(per_batch[b])
        m.update(w)
        m.update(vecs)
        m.update(brows)
        in_maps.append(m)
    return in_maps


def assemble(per_core_out: list) -> np.ndarray:
    blocks = [np.asarray(o, np.float32) for o in per_core_out]
    full = np.concatenate(blocks, axis=0).reshape(B, S, DIM)
    return full


def kernel(**inputs) -> np.ndarray:
    from concourse.bass_utils import run_bass_kernel_spmd

    nc = _get_nc()
    in_maps = make_in_maps(inputs)
    res = run_bass_kernel_spmd(nc, in_maps, list(range(N_CORES)))
    return assemble([r["out"] for r in res.results])


# revision 26
# speedup vs baseline: 1.7278x; 1.1729x over previous
"""Trainium2 Bass kernel for a dense transformer encoder layer (v2).

Model (faithful to the oracle):
  q,k,v = x@wq+bq, x@wk+bk, x@wv+bv          (12 heads, dk=64, DIM=768)
  scores = q@k^T / sqrt(768)  (note: sqrt(dim_model), not sqrt(dk))
  scores[mask==0] = 1e-11  (NOT -inf; masked keys still contribute ~1/Z)
  attn = softmax(scores); z = attn@v; o = z@wo+bo
  l1 = x + LN(o);  ffn = relu(l1@w1+b1)@w2+b2;  out = l1 + LN(ffn)

Sharding: 4096 tokens (B=2,S=2048) split 8 ways -> 512 tokens/core.
Cores 0-3 own batch 0, cores 4-7 batch 1. No collectives: each core
computes K/V itself — but only for the UNMASKED keys of its batch.

Mask compaction: masked keys (~half) all get score 1e-11, i.e. exp==1
(fp32), for every query/head. Their attention contribution is a
query-independent constant: C_h = sum_masked v_k (numerator) and
m = #masked (denominator). The host compacts unmasked tokens into a
padded [KC] buffer (KC=1280 >> max plausible count), computes the tiny
C correction in numpy, and the device runs attention only over the
compacted keys, seeding the attn@v PSUM accumulation with C via a K=1
matmul. Padded key slots have K=V=0 exactly (biases are added inside
the matmuls via a 0/1 validity row), so exp(score)=1 * V=0 contributes
nothing; the validity row also zeroes their denominator entry.

Softmax: scores are built k-major (scoresT [kpos, q]); all compacted
keys are unmasked so exp(ISCALE * s) uses a constant scale. The
denominator comes from a ones column in V (1 for real keys, 0 for
pads, via one replicated DMA); normalization happens after attn@v via
a rank-1 matmul broadcast of the fast reciprocal (input staged to SBUF
— the bit-trick op misreads PSUM).
"""

import math
import os
import sys

import numpy as np

for _p in ("/opt/trn_rl_repo", os.path.expanduser("~/.axon_site/_ro/trn_rl_repo")):
    if os.path.isdir(_p) and _p not in sys.path:
        sys.path.insert(0, _p)

import ml_dtypes  # noqa: E402

BF16 = ml_dtypes.bfloat16

DIM = 768
HEADS = 12
DK = 64
HID = 4 * DIM  # 3072
B, S = 2, 2048
N_CORES = 8
BLK = 512            # tokens per core
NBLK = S // BLK      # 4 blocks per batch
KC = 1280            # compacted-key capacity (10 tiles; ~11 sigma above E[n_u])
EPS = 1e-5
ISCALE = 1.0 / math.sqrt(DIM)

FT = DIM // 128   # 6 feature tiles
TT = BLK // 128   # 4 token tiles per core block
STC = KC // 128   # 10 compacted key tiles
HT = HID // 128   # 24 hidden tiles

_CACHE: dict = {}
MAX_PHASE = int(os.environ.get("BASS_KERNEL_PHASES", "5"))


def _build_program():
    import concourse.bass as bass
    import concourse.mybir as mybir
    import concourse.tile as tile
    from concourse import bacc
    from concourse.masks import make_identity

    f32 = mybir.dt.float32
    bf16 = mybir.dt.bfloat16
    AF = mybir.ActivationFunctionType
    ALU = mybir.AluOpType
    AX = mybir.AxisListType

    nc = bacc.Bacc()

    # ---- per-core DRAM I/O ----
    d_xTb = nc.dram_tensor("xTb", [DIM, BLK], bf16, kind="ExternalInput")
    d_xb = nc.dram_tensor("xb", [BLK, DIM], f32, kind="ExternalInput")
    d_xkvT = nc.dram_tensor("xkvT", [DIM, KC], bf16, kind="ExternalInput")
    d_onesc = nc.dram_tensor("onesc", [KC], bf16, kind="ExternalInput")
    d_onescv = nc.dram_tensor("onescv", [STC, HEADS, 128], bf16, kind="ExternalInput")
    d_wq = nc.dram_tensor("wq", [DIM, DIM], bf16, kind="ExternalInput")
    d_wk = nc.dram_tensor("wk", [DIM, DIM], bf16, kind="ExternalInput")
    d_wv = nc.dram_tensor("wv", [DIM, DIM], bf16, kind="ExternalInput")
    d_wo = nc.dram_tensor("wo", [DIM, DIM], bf16, kind="ExternalInput")
    d_w1 = nc.dram_tensor("w1", [DIM, HID], bf16, kind="ExternalInput")
    d_w2 = nc.dram_tensor("w2", [HID, DIM], bf16, kind="ExternalInput")
    d_bkrow = nc.dram_tensor("bkrow", [DIM], bf16, kind="ExternalInput")
    d_bvrow = nc.dram_tensor("bvrow", [DIM], bf16, kind="ExternalInput")
    d_bqrow = nc.dram_tensor("bqrow", [DIM], bf16, kind="ExternalInput")
    d_crow = nc.dram_tensor("crow", [HEADS * (DK + 1)], f32, kind="ExternalInput")
    d_bo = nc.dram_tensor("bo", [DIM], f32, kind="ExternalInput")
    d_b1 = nc.dram_tensor("b1", [HID], f32, kind="ExternalInput")
    d_b2 = nc.dram_tensor("b2", [DIM], f32, kind="ExternalInput")
    d_g1 = nc.dram_tensor("g1", [DIM], f32, kind="ExternalInput")
    d_bb1 = nc.dram_tensor("bb1", [DIM], f32, kind="ExternalInput")
    d_g2 = nc.dram_tensor("g2", [DIM], f32, kind="ExternalInput")
    d_bb2 = nc.dram_tensor("bb2", [DIM], f32, kind="ExternalInput")
    d_out = nc.dram_tensor("out", [BLK, DIM], f32, kind="ExternalOutput")

    KCH = [(0, 512), (512, 1024), (1024, KC)]  # K-proj N chunks

    def bcast_ap(handle, n=128):
        ap = handle[:]
        return bass.AP(tensor=ap.tensor, offset=ap.offset, ap=[[0, n]] + list(ap.ap))

    with tile.TileContext(nc) as tc:
        with (
            tc.tile_pool(name="const", bufs=1) as const,
            tc.tile_pool(name="bigres", bufs=1) as big,
        ):
            # ---------- constants ----------
            # (row/bias constants ride the gpsimd queue or late sync slots;
            # the sync queue head is reserved for phase-1-critical loads)
            sb_b1 = const.tile([128, HT], f32)
            bo_bc = const.tile([128, DIM], bf16)
            nc.gpsimd.dma_start(out=bo_bc, in_=bcast_ap(d_bo))
            b2_bc = const.tile([128, DIM], bf16)
            nc.gpsimd.dma_start(out=b2_bc, in_=bcast_ap(d_b2))
            g1_bc = const.tile([128, DIM], bf16)
            nc.gpsimd.dma_start(out=g1_bc, in_=bcast_ap(d_g1))
            bb1_bc = const.tile([128, DIM], bf16)
            nc.gpsimd.dma_start(out=bb1_bc, in_=bcast_ap(d_bb1))
            g2_bc = const.tile([128, DIM], bf16)
            nc.gpsimd.dma_start(out=g2_bc, in_=bcast_ap(d_g2))
            bb2_bc = const.tile([128, DIM], bf16)
            nc.gpsimd.dma_start(out=bb2_bc, in_=bcast_ap(d_bb2))
            ident = const.tile([128, 128], f32)
            make_identity(nc, ident[:])
            ones64 = const.tile([1, 64], f32)
            nc.vector.memset(ones64, 1.0)
            ones512f = const.tile([1, BLK], f32)
            nc.vector.memset(ones512f, 1.0)
            ones512b = const.tile([1, BLK], bf16)
            nc.vector.memset(ones512b, 1.0)
            eps_t = const.tile([128, 1], f32)
            nc.vector.memset(eps_t, EPS)
            sb_bkrow = const.tile([1, DIM], bf16)
            sb_bvrow = const.tile([1, DIM], bf16)
            sb_bqrow = const.tile([1, DIM], bf16)
            sb_crow = const.tile([1, HEADS * (DK + 1)], f32)
            sb_onesc = const.tile([1, KC], bf16)

            # ---------- persistent activations ----------
            sb_xblk = big.tile([128, TT, DIM], f32)  # residual x
            sb_l1 = big.tile([128, TT, DIM], f32)

            # weight-prefetch pool: opened before attn_res so pool pops
            # stay LIFO (attn_res dies first); DMAs are issued after phase 1
            wpre_cm = tc.tile_pool(name="wpre", bufs=1)
            wpre = wpre_cm.__enter__()
            w_o = wpre.tile([128, FT, DIM], bf16)
            w1_t = []
            for kt in range(FT):
                w1_kt = wpre.tile([128, HID], bf16, tag=f"w1_{kt}", name=f"w1_{kt}")
                w1_t.append(w1_kt)

            # attention-scoped residents (freed before the FFN phases)
            attn_res_cm = tc.tile_pool(name="attn_res", bufs=1)
            attn_res = attn_res_cm.__enter__()
            sb_K = attn_res.tile([128, FT, KC], bf16)  # K^T, feat-major
            sb_Q = attn_res.tile([128, FT, BLK], bf16)  # Q^T, feat-major
            sb_V = attn_res.tile([128, STC, HEADS, DK + 1], bf16)  # V + ones col
            sb_zT = attn_res.tile([128, FT, BLK], bf16)  # z^T normalized

            # ones column of V: validity row, host-replicated across heads
            nc.gpsimd.dma_start(
                out=sb_V[:, :, :, DK : DK + 1],
                in_=d_onescv[:].rearrange("t h p -> p t h"),
            )

            # ============ Phase 1: QKV projections ============
            with (
                tc.tile_pool(name="xw", bufs=1) as xw,
                tc.tile_pool(name="ps1", bufs=4, space="PSUM") as ps1,
                tc.tile_pool(name="ps1v", bufs=4, space="PSUM") as ps1v,
            ):
                # sync-queue order = need order: K-proj inputs first (per-kt
                # slices so the first psum chain starts as soon as slice 0
                # lands), then Q/V inputs, then phase-3+ constants.
                sb_xkvT = xw.tile([128, FT, KC], bf16)
                w_k = xw.tile([128, FT, DIM], bf16)
                for kt in range(FT):
                    nc.sync.dma_start(
                        out=sb_xkvT[:, kt, :],
                        in_=d_xkvT[kt * 128 : (kt + 1) * 128, :],
                    )
                    nc.sync.dma_start(
                        out=w_k[:, kt, :], in_=d_wk[kt * 128 : (kt + 1) * 128, :]
                    )
                nc.sync.dma_start(out=sb_bkrow, in_=bcast_ap(d_bkrow, 1))
                nc.sync.dma_start(out=sb_onesc, in_=bcast_ap(d_onesc, 1))
                sb_xTb = xw.tile([128, FT, BLK], bf16)
                nc.sync.dma_start(
                    out=sb_xTb, in_=d_xTb[:].rearrange("(t p) n -> p t n", p=128)
                )
                nc.sync.dma_start(out=sb_bqrow, in_=bcast_ap(d_bqrow, 1))
                w_q = xw.tile([128, FT, DIM], bf16)
                for kt in range(FT):
                    nc.sync.dma_start(
                        out=w_q[:, kt, :], in_=d_wq[kt * 128 : (kt + 1) * 128, :]
                    )
                w_v = xw.tile([128, FT, DIM], bf16)
                for kt in range(FT):
                    nc.sync.dma_start(
                        out=w_v[:, kt, :], in_=d_wv[kt * 128 : (kt + 1) * 128, :]
                    )
                nc.sync.dma_start(out=sb_bvrow, in_=bcast_ap(d_bvrow, 1))
                nc.sync.dma_start(out=sb_crow, in_=bcast_ap(d_crow, 1))
                nc.sync.dma_start(
                    out=sb_xblk, in_=d_xb[:].rearrange("(t p) d -> p t d", p=128)
                )
                nc.sync.dma_start(
                    out=sb_b1, in_=d_b1[:].rearrange("(t p) -> p t", p=128)
                )

                # K^T feat-major over compacted keys; bias via validity row
                for ft in range(FT):
                    for c0, c1 in KCH:
                        ps = ps1.tile([128, c1 - c0], f32, tag="p")
                        for kt in range(FT):
                            nc.tensor.matmul(
                                ps,
                                w_k[:, kt, ft * 128 : (ft + 1) * 128],
                                sb_xkvT[:, kt, c0:c1],
                                start=(kt == 0),
                                stop=False,
                            )
                        nc.tensor.matmul(
                            ps,
                            sb_bkrow[0:1, ft * 128 : (ft + 1) * 128],
                            sb_onesc[0:1, c0:c1],
                            start=False,
                            stop=True,
                        )
                        nc.scalar.copy(sb_K[:, ft, c0:c1], ps)
                # Q^T feat-major for the core's block
                for ft in range(FT):
                    ps = ps1.tile([128, BLK], f32, tag="p")
                    for kt in range(FT):
                        nc.tensor.matmul(
                            ps,
                            w_q[:, kt, ft * 128 : (ft + 1) * 128],
                            sb_xTb[:, kt, :],
                            start=(kt == 0),
                            stop=False,
                        )
                    nc.tensor.matmul(
                        ps,
                        sb_bqrow[0:1, ft * 128 : (ft + 1) * 128],
                        ones512b[:],
                        start=False,
                        stop=True,
                    )
                    nc.scalar.copy(sb_Q[:, ft, :], ps)
                # V tok-major over compacted keys, [tok, head, dk]
                for nh in range(2):
                    for tt in range(STC):
                        ps = ps1v.tile([128, 384], f32, tag="vp")
                        for kt in range(FT):
                            nc.tensor.matmul(
                                ps,
                                sb_xkvT[:, kt, tt * 128 : (tt + 1) * 128],
                                w_v[:, kt, nh * 384 : (nh + 1) * 384],
                                start=(kt == 0),
                                stop=False,
                            )
                        nc.tensor.matmul(
                            ps,
                            sb_onesc[0:1, tt * 128 : (tt + 1) * 128],
                            sb_bvrow[0:1, nh * 384 : (nh + 1) * 384],
                            start=False,
                            stop=True,
                        )
                        nc.vector.tensor_copy(
                            sb_V[:, tt, nh * 6 : (nh + 1) * 6, 0:DK],
                            ps[:].rearrange("p (h d) -> p h d", d=DK),
                        )

            # prefetched O-proj / FFN1 weights: DMAs are issued after the
            # phase-1 loads (below) and so overlap attention instead of
            # stalling behind it
            for kt in range(FT):
                nc.sync.dma_start(
                    out=w_o[:, kt, :], in_=d_wo[kt * 128 : (kt + 1) * 128, :]
                )
            for kt in range(FT):
                nc.sync.dma_start(
                    out=w1_t[kt], in_=d_w1[kt * 128 : (kt + 1) * 128, :]
                )

            if MAX_PHASE >= 2:
                # ============ Phase 2: attention ============
                with (
                    tc.tile_pool(name="expp", bufs=20) as expp,
                    tc.tile_pool(name="attsm", bufs=2) as attsm,
                    tc.tile_pool(name="ps_sc", bufs=2, space="PSUM") as ps_sc,
                    tc.tile_pool(name="ps_z", bufs=1, space="PSUM") as ps_z,
                    tc.tile_pool(name="ps_rb", bufs=2, space="PSUM") as ps_rb,
                ):
                    for hp in range(HEADS // 2):
                        ht = hp
                        # interleave the two heads of a pair kt-by-kt: their
                        # K=64 matmuls sit in disjoint PE row groups (0-63 /
                        # 64-127) so the hardware overlaps adjacent pairs.
                        # Both halves land in one 2-bank PSUM tile so a single
                        # [128,1024] exp covers them (halves ACT dispatch+sem
                        # overhead).
                        ets = ([], [])
                        for kt2 in range(STC):
                            ps = ps_sc.tile([128, 2 * BLK], f32, tag="sc")
                            for half in (0, 1):
                                ho = half * 64
                                nc.tensor.matmul(
                                    ps[:, half * BLK : (half + 1) * BLK],
                                    sb_K[ho : ho + 64, ht, kt2 * 128 : (kt2 + 1) * 128],
                                    sb_Q[ho : ho + 64, ht, :],
                                    start=True,
                                    stop=True,
                                )
                            et = expp.tile([128, 2 * BLK], bf16, tag="exp")
                            nc.scalar.activation(et, ps, AF.Exp, scale=ISCALE)
                            for half in (0, 1):
                                ets[half].append(
                                    et[:, half * BLK : (half + 1) * BLK]
                                )
                        zps = []
                        for half in (0, 1):
                            h = 2 * hp + half
                            zp = ps_z.tile([DK + 1, BLK], f32, tag=f"z{half}")
                            # seed with the masked-keys correction row
                            nc.tensor.matmul(
                                zp,
                                sb_crow[0:1, h * (DK + 1) : (h + 1) * (DK + 1)],
                                ones512f[:],
                                start=True,
                                stop=False,
                            )
                            for kt2 in range(STC):
                                nc.tensor.matmul(
                                    zp,
                                    sb_V[:, kt2, h, :],
                                    ets[half][kt2],
                                    start=False,
                                    stop=(kt2 == STC - 1),
                                )
                            zps.append(zp)
                        for half in (0, 1):
                            ho = half * 64
                            # denominators are huge sums (>= 1): the ~18-bit
                            # fast reciprocal is ~5x cheaper and plenty exact.
                            # (bitwise-trick op: input must be in SBUF, not PSUM)
                            zrow = attsm.tile([1, BLK], f32, tag="zrow")
                            nc.vector.tensor_copy(zrow, zps[half][DK : DK + 1, :])
                            rsum = attsm.tile([1, BLK], f32, tag="rsum")
                            nc.vector.reciprocal_approx_fast(rsum, zrow)
                            rbp = ps_rb.tile([64, BLK], f32, tag="rb")
                            nc.tensor.matmul(
                                rbp, ones64[:], rsum, start=True, stop=True
                            )
                            rb = attsm.tile([64, BLK], f32, tag="rbs")
                            nc.vector.tensor_copy(rb, rbp)
                            nc.vector.tensor_mul(
                                sb_zT[ho : ho + 64, ht, :], zps[half][0:DK, :], rb
                            )

            if MAX_PHASE >= 3:
                # ============ Phase 3: O proj + LN1 (+residual) ============
                def layer_norm_to(out_ap, x_ap, g_bc_t, resid_ap, pool):
                    s = pool.tile([128, 1], f32, tag="ln_s")
                    nc.vector.tensor_reduce(s, x_ap, axis=AX.X, op=ALU.add)
                    junk = pool.tile([128, DIM], f32, tag="ln_j")
                    ssq = pool.tile([128, 1], f32, tag="ln_q")
                    # (tensor_tensor_reduce crashes the device on this runtime;
                    # scalar_tensor_tensor with accum_out works)
                    nc.vector.scalar_tensor_tensor(
                        out=junk, in0=x_ap, scalar=1.0, in1=x_ap,
                        op0=ALU.mult, op1=ALU.mult, accum_out=ssq,
                    )
                    negmean = pool.tile([128, 1], f32, tag="ln_m")
                    nc.scalar.mul(negmean, s, -1.0 / DIM)
                    # var = E[x^2] - mean^2
                    m2 = pool.tile([128, 1], f32, tag="ln_m2")
                    nc.vector.tensor_mul(m2, negmean, negmean)
                    var = pool.tile([128, 1], f32, tag="ln_v")
                    nc.vector.scalar_tensor_tensor(
                        out=var, in0=ssq, scalar=1.0 / DIM, in1=m2,
                        op0=ALU.mult, op1=ALU.subtract,
                    )
                    sd = pool.tile([128, 1], f32, tag="ln_sd")
                    nc.scalar.activation(sd, var, AF.Sqrt, bias=eps_t[:])
                    rstd = pool.tile([128, 1], f32, tag="ln_r")
                    nc.vector.reciprocal(rstd, sd)
                    nmr = pool.tile([128, 1], f32, tag="ln_nm")
                    nc.vector.tensor_mul(nmr, negmean, rstd)
                    # x*rstd on ACT; then (x*rstd - mu*rstd) * g fused on DVE
                    nrm = pool.tile([128, DIM], f32, tag="ln_t")
                    nc.scalar.mul(nrm, x_ap, rstd[:])
                    tg = pool.tile([128, DIM], f32, tag="ln_tg")
                    nc.vector.scalar_tensor_tensor(
                        out=tg, in0=nrm, scalar=nmr[:], in1=g_bc_t,
                        op0=ALU.add, op1=ALU.mult,
                    )
                    nc.vector.tensor_add(out_ap, tg, resid_ap)

                with (
                    tc.tile_pool(name="ln1p", bufs=2) as ln1p,
                    tc.tile_pool(name="ps_o", bufs=4, space="PSUM") as ps_o,
                ):
                    for tt in range(TT):
                        l1pre = ln1p.tile([128, DIM], f32, tag="l1pre")
                        for nh in range(2):
                            ps = ps_o.tile([128, 384], f32, tag="op")
                            for kt in range(FT):
                                nc.tensor.matmul(
                                    ps,
                                    sb_zT[:, kt, tt * 128 : (tt + 1) * 128],
                                    w_o[:, kt, nh * 384 : (nh + 1) * 384],
                                    start=(kt == 0),
                                    stop=(kt == FT - 1),
                                )
                            nc.vector.scalar_tensor_tensor(
                                out=l1pre[:, nh * 384 : (nh + 1) * 384],
                                in0=ps,
                                scalar=1.0,
                                in1=bo_bc[:, nh * 384 : (nh + 1) * 384],
                                op0=ALU.mult,
                                op1=ALU.add,
                            )
                        xb1 = ln1p.tile([128, DIM], f32, tag="xb1")
                        nc.vector.tensor_add(xb1, sb_xblk[:, tt, :], bb1_bc)
                        layer_norm_to(sb_l1[:, tt, :], l1pre[:], g1_bc, xb1, ln1p)

            attn_res_cm.__exit__(None, None, None)
            hT_cm = tc.tile_pool(name="hTp", bufs=1)
            hTp = hT_cm.__enter__()
            sb_hT = hTp.tile([128, HT, BLK], bf16)  # relu(ffn1)^T, hid-major

            # prefetch FFN2 weights during FFN1 (attention pools are gone,
            # so the space is free and the DMA overlaps transposes/FFN1)
            w2pre_cm = tc.tile_pool(name="w2pre", bufs=1)
            w2pre = w2pre_cm.__enter__()
            w2_t = []
            for kt in range(HT):
                wt = w2pre.tile([128, DIM], bf16, tag=f"w2_{kt}")
                nc.sync.dma_start(out=wt, in_=d_w2[kt * 128 : (kt + 1) * 128, :])
                w2_t.append(wt)

            if MAX_PHASE >= 4:
                # ============ Phase 4: transpose l1, FFN1 ============
                with (
                    tc.tile_pool(name="l1t_p", bufs=1) as l1t_p,
                    tc.tile_pool(name="ps_t", bufs=2, space="PSUM") as ps_t,
                    tc.tile_pool(name="ps_f1", bufs=4, space="PSUM") as ps_f1,
                ):
                    sb_l1T = l1t_p.tile([128, FT, BLK], bf16)
                    for ft in range(FT):
                        for tt in range(TT):
                            pst = ps_t.tile([128, 128], f32, tag="tp")
                            nc.tensor.transpose(
                                pst, sb_l1[:, tt, ft * 128 : (ft + 1) * 128], ident[:]
                            )
                            nc.scalar.copy(
                                sb_l1T[:, ft, tt * 128 : (tt + 1) * 128], pst
                            )
                    for ht2 in range(HT):
                        ps = ps_f1.tile([128, BLK], f32, tag="f1")
                        for kt in range(FT):
                            nc.tensor.matmul(
                                ps,
                                w1_t[kt][:, ht2 * 128 : (ht2 + 1) * 128],
                                sb_l1T[:, kt, :],
                                start=(kt == 0),
                                stop=(kt == FT - 1),
                            )
                        # relu(x + b1) on DVE: (x add b1) max 0
                        nc.vector.tensor_scalar(
                            sb_hT[:, ht2, :], ps, sb_b1[:, ht2 : ht2 + 1], 0.0,
                            op0=ALU.add, op1=ALU.max,
                        )

            if MAX_PHASE >= 5:
                # ============ Phase 5: FFN2 + LN2 + out ============
                with (
                    tc.tile_pool(name="ln2p", bufs=2) as ln2p,
                    tc.tile_pool(name="outp", bufs=3) as outp,
                    tc.tile_pool(name="ps_f2", bufs=4, space="PSUM") as ps_f2,
                ):
                    out_r = d_out[:].rearrange("(t p) d -> p t d", p=128)
                    for tt in range(TT):
                        f2pre = ln2p.tile([128, DIM], f32, tag="f2pre")
                        for nh in range(2):
                            ps = ps_f2.tile([128, 384], f32, tag="f2")
                            for kt in range(HT):
                                nc.tensor.matmul(
                                    ps,
                                    sb_hT[:, kt, tt * 128 : (tt + 1) * 128],
                                    w2_t[kt][:, nh * 384 : (nh + 1) * 384],
                                    start=(kt == 0),
                                    stop=(kt == HT - 1),
                                )
                            nc.vector.scalar_tensor_tensor(
                                out=f2pre[:, nh * 384 : (nh + 1) * 384],
                                in0=ps,
                                scalar=1.0,
                                in1=b2_bc[:, nh * 384 : (nh + 1) * 384],
                                op0=ALU.mult,
                                op1=ALU.add,
                            )
                        l1b = ln2p.tile([128, DIM], f32, tag="l1b")
                        nc.vector.tensor_add(l1b, sb_l1[:, tt, :], bb2_bc)
                        o_sb = outp.tile([128, DIM], f32, tag="osb")
                        layer_norm_to(o_sb[:], f2pre[:], g2_bc, l1b, ln2p)
                        nc.sync.dma_start(out=out_r[:, tt, :], in_=o_sb)

            w2pre_cm.__exit__(None, None, None)
            hT_cm.__exit__(None, None, None)
            wpre_cm.__exit__(None, None, None)

    return nc


def _get_nc(finalized=True):
    if "nc" not in _CACHE:
        _CACHE["nc"] = _build_program()
    nc = _CACHE["nc"]
    if finalized and not nc.is_finalized():
        nc.finalize()
    return nc


def make_in_maps(inputs: dict) -> list:
    x = np.asarray(inputs["x_n"], np.float32).reshape(B, S, DIM)
    mask = np.asarray(inputs["mask"]).reshape(B, S)
    w = {
        k: np.ascontiguousarray(np.asarray(inputs[k], np.float32).astype(BF16))
        for k in ("wq", "wk", "wv", "wo", "w1", "w2")
    }
    vecs = {
        "bo": inputs["bo"], "b1": inputs["b1"], "b2": inputs["b2"],
        "g1": inputs["ln1_g"], "bb1": inputs["ln1_b"],
        "g2": inputs["ln2_g"], "bb2": inputs["ln2_b"],
    }
    vecs = {k: np.ascontiguousarray(np.asarray(v, np.float32)) for k, v in vecs.items()}
    brows = {
        "bkrow": np.asarray(inputs["bk"], np.float32).astype(BF16),
        "bvrow": np.asarray(inputs["bv"], np.float32).astype(BF16),
        "bqrow": np.asarray(inputs["bq"], np.float32).astype(BF16),
    }

    # per-batch compaction + masked-keys correction
    per_batch = []
    for b in range(B):
        mb = mask[b] != 0
        idx = np.nonzero(mb)[0]
        n_u = len(idx)
        if n_u > KC:
            raise RuntimeError(
                f"unmasked key count {n_u} exceeds compiled capacity {KC}"
            )
        xkv = np.zeros((KC, DIM), np.float32)
        xkv[:n_u] = x[b][idx]
        xkvT = np.ascontiguousarray(xkv.T.astype(BF16))
        onesc = np.zeros(KC, np.float32)
        onesc[:n_u] = 1.0
        msum = x[b][~mb].astype(np.float64).sum(axis=0)
        mcount = float((~mb).sum())
        wv64 = np.asarray(inputs["wv"], np.float64)
        bv64 = np.asarray(inputs["bv"], np.float64)
        cvec = (msum @ wv64 + mcount * bv64).astype(np.float32)  # [DIM]
        crow = np.zeros(HEADS * (DK + 1), np.float32)
        ch = cvec.reshape(HEADS, DK)
        for h in range(HEADS):
            crow[h * (DK + 1) : h * (DK + 1) + DK] = ch[h]
            crow[h * (DK + 1) + DK] = mcount
        onesc_bf = onesc.astype(BF16)
        onescv = np.ascontiguousarray(
            np.broadcast_to(
                onesc_bf.reshape(STC, 1, 128), (STC, HEADS, 128)
            )
        )
        per_batch.append(
            {"xkvT": xkvT, "onesc": onesc_bf, "onescv": onescv, "crow": crow}
        )

    in_maps = []
    for c in range(N_CORES):
        b, blk = c // NBLK, c % NBLK
        xb = x[b]
        xblk = np.ascontiguousarray(xb[blk * BLK : (blk + 1) * BLK])
        xTb = np.ascontiguousarray(xblk.T.astype(BF16))
        m = {"xTb": xTb, "xb": xblk}
        m.update(per_batch[b])
        m.update(w)
        m.update(vecs)
        m.update(brows)
        in_maps.append(m)
    return in_maps


def assemble(per_core_out: list) -> np.ndarray:
    blocks = [np.asarray(o, np.float32) for o in per_core_out]
    full = np.concatenate(blocks, axis=0).reshape(B, S, DIM)
    return full


def kernel(**inputs) -> np.ndarray:
    from concourse.bass_utils import run_bass_kernel_spmd

    nc = _get_nc()
    in_maps = make_in_maps(inputs)
    res = run_bass_kernel_spmd(nc, in_maps, list(range(N_CORES)))
    return assemble([r["out"] for r in res.results])


# revision 30
# speedup vs baseline: 1.8376x; 1.0635x over previous
"""Trainium2 Bass kernel for a dense transformer encoder layer (v2).

Model (faithful to the oracle):
  q,k,v = x@wq+bq, x@wk+bk, x@wv+bv          (12 heads, dk=64, DIM=768)
  scores = q@k^T / sqrt(768)  (note: sqrt(dim_model), not sqrt(dk))
  scores[mask==0] = 1e-11  (NOT -inf; masked keys still contribute ~1/Z)
  attn = softmax(scores); z = attn@v; o = z@wo+bo
  l1 = x + LN(o);  ffn = relu(l1@w1+b1)@w2+b2;  out = l1 + LN(ffn)

Sharding: 4096 tokens (B=2,S=2048) split 8 ways -> 512 tokens/core.
Cores 0-3 own batch 0, cores 4-7 batch 1. No collectives: each core
computes K/V itself — but only for the UNMASKED keys of its batch.

Mask compaction: masked keys (~half) all get score 1e-11, i.e. exp==1
(fp32), for every query/head. Their attention contribution is a
query-independent constant: C_h = sum_masked v_k (numerator) and
m = #masked (denominator). The host compacts unmasked tokens into a
padded [KC] buffer (KC=1280 >> max plausible count), computes the tiny
C correction in numpy, and the device runs attention only over the
compacted keys, seeding the attn@v PSUM accumulation with C via a K=1
matmul. Padded key slots have K=V=0 exactly (biases are added inside
the matmuls via a 0/1 validity row), so exp(score)=1 * V=0 contributes
nothing; the validity row also zeroes their denominator entry.

Softmax: scores are built k-major (scoresT [kpos, q]); all compacted
keys are unmasked so exp(ISCALE * s) uses a constant scale. The
denominator comes from a ones column in V (1 for real keys, 0 for
pads, via one replicated DMA); normalization happens after attn@v via
a rank-1 matmul broadcast of the fast reciprocal (input staged to SBUF
— the bit-trick op misreads PSUM).
"""

import math
import os
import sys

import numpy as np

for _p in ("/opt/trn_rl_repo", os.path.expanduser("~/.axon_site/_ro/trn_rl_repo")):
    if os.path.isdir(_p) and _p not in sys.path:
        sys.path.insert(0, _p)

import ml_dtypes  # noqa: E402

BF16 = ml_dtypes.bfloat16

DIM = 768
HEADS = 12
DK = 64
HID = 4 * DIM  # 3072
B, S = 2, 2048
N_CORES = 8
BLK = 512            # tokens per core
NBLK = S // BLK      # 4 blocks per batch
KC = 1152            # compacted-key capacity (9 tiles; ~5.7 sigma above E[n_u])
EPS = 1e-5
ISCALE = 1.0 / math.sqrt(DIM)

FT = DIM // 128   # 6 feature tiles
TT = BLK // 128   # 4 token tiles per core block
STC = KC // 128   # 10 compacted key tiles
HT = HID // 128   # 24 hidden tiles

_CACHE: dict = {}
MAX_PHASE = int(os.environ.get("BASS_KERNEL_PHASES", "5"))


def _build_program():
    import concourse.bass as bass
    import concourse.mybir as mybir
    import concourse.tile as tile
    from concourse import bacc
    from concourse.masks import make_identity

    f32 = mybir.dt.float32
    bf16 = mybir.dt.bfloat16
    AF = mybir.ActivationFunctionType
    ALU = mybir.AluOpType
    AX = mybir.AxisListType

    nc = bacc.Bacc()

    # ---- per-core DRAM I/O ----
    d_xTb = nc.dram_tensor("xTb", [DIM, BLK], bf16, kind="ExternalInput")
    d_xb = nc.dram_tensor("xb", [BLK, DIM], f32, kind="ExternalInput")
    d_xkvT = nc.dram_tensor("xkvT", [DIM, KC], bf16, kind="ExternalInput")
    d_onesc = nc.dram_tensor("onesc", [KC], bf16, kind="ExternalInput")
    d_wq = nc.dram_tensor("wq", [DIM, DIM], bf16, kind="ExternalInput")
    d_wk = nc.dram_tensor("wk", [DIM, DIM], bf16, kind="ExternalInput")
    d_wv = nc.dram_tensor("wv", [DIM, DIM], bf16, kind="ExternalInput")
    d_wo = nc.dram_tensor("wo", [DIM, DIM], bf16, kind="ExternalInput")
    d_w1 = nc.dram_tensor("w1", [DIM, HID], bf16, kind="ExternalInput")
    d_w2 = nc.dram_tensor("w2", [HID, DIM], bf16, kind="ExternalInput")
    d_bkrow = nc.dram_tensor("bkrow", [DIM], bf16, kind="ExternalInput")
    d_bvrow = nc.dram_tensor("bvrow", [DIM], bf16, kind="ExternalInput")
    d_bqrow = nc.dram_tensor("bqrow", [DIM], bf16, kind="ExternalInput")
    d_crow = nc.dram_tensor("crow", [HEADS * (DK + 1)], f32, kind="ExternalInput")
    d_bo = nc.dram_tensor("bo", [DIM], f32, kind="ExternalInput")
    d_b1 = nc.dram_tensor("b1", [HID], f32, kind="ExternalInput")
    d_b2 = nc.dram_tensor("b2", [DIM], f32, kind="ExternalInput")
    d_g1 = nc.dram_tensor("g1", [DIM], f32, kind="ExternalInput")
    d_bb1 = nc.dram_tensor("bb1", [DIM], f32, kind="ExternalInput")
    d_g2 = nc.dram_tensor("g2", [DIM], f32, kind="ExternalInput")
    d_bb2 = nc.dram_tensor("bb2", [DIM], f32, kind="ExternalInput")
    d_out = nc.dram_tensor("out", [BLK, DIM], f32, kind="ExternalOutput")

    KCH = [(0, 512), (512, 1024), (1024, KC)]  # K-proj N chunks

    def bcast_ap(handle, n=128):
        ap = handle[:]
        return bass.AP(tensor=ap.tensor, offset=ap.offset, ap=[[0, n]] + list(ap.ap))

    with tile.TileContext(nc) as tc:
        with (
            tc.tile_pool(name="const", bufs=1) as const,
            tc.tile_pool(name="bigres", bufs=1) as big,
        ):
            # ---------- constants ----------
            # (row/bias constants ride the gpsimd queue or late sync slots;
            # the sync queue head is reserved for phase-1-critical loads)
            sb_b1 = const.tile([128, HT], f32)
            bo_bc = const.tile([128, DIM], bf16)
            nc.gpsimd.dma_start(out=bo_bc, in_=bcast_ap(d_bo))
            b2_bc = const.tile([128, DIM], bf16)
            nc.gpsimd.dma_start(out=b2_bc, in_=bcast_ap(d_b2))
            g1_bc = const.tile([128, DIM], bf16)
            nc.gpsimd.dma_start(out=g1_bc, in_=bcast_ap(d_g1))
            bb1_bc = const.tile([128, DIM], bf16)
            nc.gpsimd.dma_start(out=bb1_bc, in_=bcast_ap(d_bb1))
            g2_bc = const.tile([128, DIM], bf16)
            nc.gpsimd.dma_start(out=g2_bc, in_=bcast_ap(d_g2))
            bb2_bc = const.tile([128, DIM], bf16)
            nc.gpsimd.dma_start(out=bb2_bc, in_=bcast_ap(d_bb2))
            ident = const.tile([128, 128], f32)
            make_identity(nc, ident[:])
            ones64 = const.tile([1, 64], f32)
            nc.vector.memset(ones64, 1.0)
            ones512f = const.tile([1, BLK], f32)
            nc.vector.memset(ones512f, 1.0)
            ones512b = const.tile([1, BLK], bf16)
            nc.vector.memset(ones512b, 1.0)
            eps_t = const.tile([128, 1], f32)
            nc.vector.memset(eps_t, EPS)
            sb_bkrow = const.tile([1, DIM], bf16)
            sb_bvrow = const.tile([1, DIM], bf16)
            sb_bqrow = const.tile([1, DIM], bf16)
            sb_crow = const.tile([1, HEADS * (DK + 1)], f32)
            sb_onesc = const.tile([1, KC], bf16)
            sb_onescT = const.tile([128, STC], bf16)

            # ---------- persistent activations ----------
            sb_xblk = big.tile([128, TT, DIM], f32)  # residual x
            sb_l1 = big.tile([128, TT, DIM], f32)

            # weight-prefetch pool: opened before attn_res so pool pops
            # stay LIFO (attn_res dies first); DMAs are issued after phase 1
            wpre_cm = tc.tile_pool(name="wpre", bufs=1)
            wpre = wpre_cm.__enter__()
            w_o = wpre.tile([128, FT, DIM], bf16)
            w1_t = []
            for kt in range(FT):
                w1_kt = wpre.tile([128, HID], bf16, tag=f"w1_{kt}", name=f"w1_{kt}")
                w1_t.append(w1_kt)


            # attention-scoped residents (freed before the FFN phases)
            attn_res_cm = tc.tile_pool(name="attn_res", bufs=1)
            attn_res = attn_res_cm.__enter__()
            sb_K = attn_res.tile([128, FT, KC], bf16)  # K^T, feat-major
            sb_Q = attn_res.tile([128, FT, BLK], bf16)  # Q^T, feat-major
            sb_V = attn_res.tile([128, STC, HEADS, DK + 1], bf16)  # V + ones col
            sb_zT = attn_res.tile([128, FT, BLK], bf16)  # z^T normalized


            # ============ Phase 1: QKV projections ============
            with (
                tc.tile_pool(name="xw", bufs=1) as xw,
                tc.tile_pool(name="ps1", bufs=4, space="PSUM") as ps1,
                tc.tile_pool(name="ps1v", bufs=4, space="PSUM") as ps1v,
            ):
                # sync-queue order = need order: Q-proj inputs first (PE can
                # start on Q within ~6us), then K inputs per-kt, then V,
                # then phase-3+ constants.
                sb_xTb = xw.tile([128, FT, BLK], bf16)
                nc.sync.dma_start(
                    out=sb_xTb, in_=d_xTb[:].rearrange("(t p) n -> p t n", p=128)
                )
                nc.sync.dma_start(out=sb_bqrow, in_=bcast_ap(d_bqrow, 1))
                w_q = xw.tile([128, FT, DIM], bf16)
                for kt in range(FT):
                    nc.sync.dma_start(
                        out=w_q[:, kt, :], in_=d_wq[kt * 128 : (kt + 1) * 128, :]
                    )
                sb_xkvT = xw.tile([128, FT, KC], bf16)
                w_k = xw.tile([128, FT, DIM], bf16)
                for kt in range(FT):
                    nc.sync.dma_start(
                        out=sb_xkvT[:, kt, :],
                        in_=d_xkvT[kt * 128 : (kt + 1) * 128, :],
                    )
                    nc.sync.dma_start(
                        out=w_k[:, kt, :], in_=d_wk[kt * 128 : (kt + 1) * 128, :]
                    )
                nc.sync.dma_start(out=sb_bkrow, in_=bcast_ap(d_bkrow, 1))
                nc.sync.dma_start(out=sb_onesc, in_=bcast_ap(d_onesc, 1))
                nc.sync.dma_start(
                    out=sb_onescT, in_=d_onesc[:].rearrange("(t p) -> p t", p=128)
                )
                # ones column of V: 12 cheap strided copies from the dense
                # tok-major validity tile (a direct strided DMA generates
                # 2-byte-packet descriptor spam and a ~26us drain)
                for h in range(HEADS):
                    nc.vector.tensor_copy(
                        sb_V[:, :, h, DK : DK + 1],
                        sb_onescT[:].rearrange("p (t o) -> p t o", o=1),
                    )
                w_v = xw.tile([128, FT, DIM], bf16)
                for kt in range(FT):
                    nc.sync.dma_start(
                        out=w_v[:, kt, :], in_=d_wv[kt * 128 : (kt + 1) * 128, :]
                    )
                nc.sync.dma_start(out=sb_bvrow, in_=bcast_ap(d_bvrow, 1))
                nc.sync.dma_start(out=sb_crow, in_=bcast_ap(d_crow, 1))
                nc.sync.dma_start(
                    out=sb_xblk, in_=d_xb[:].rearrange("(t p) d -> p t d", p=128)
                )
                nc.sync.dma_start(
                    out=sb_b1, in_=d_b1[:].rearrange("(t p) -> p t", p=128)
                )

                # Q^T feat-major for the core's block (first: inputs land first)
                for ft in range(FT):
                    ps = ps1.tile([128, BLK], f32, tag="p")
                    for kt in range(FT):
                        nc.tensor.matmul(
                            ps,
                            w_q[:, kt, ft * 128 : (ft + 1) * 128],
                            sb_xTb[:, kt, :],
                            start=(kt == 0),
                            stop=False,
                        )
                    nc.tensor.matmul(
                        ps,
                        sb_bqrow[0:1, ft * 128 : (ft + 1) * 128],
                        ones512b[:],
                        start=False,
                        stop=True,
                    )
                    nc.scalar.copy(sb_Q[:, ft, :], ps)
                # K^T feat-major over compacted keys; bias via validity row
                for ft in range(FT):
                    for c0, c1 in KCH:
                        ps = ps1.tile([128, c1 - c0], f32, tag="p")
                        for kt in range(FT):
                            nc.tensor.matmul(
                                ps,
                                w_k[:, kt, ft * 128 : (ft + 1) * 128],
                                sb_xkvT[:, kt, c0:c1],
                                start=(kt == 0),
                                stop=False,
                            )
                        nc.tensor.matmul(
                            ps,
                            sb_bkrow[0:1, ft * 128 : (ft + 1) * 128],
                            sb_onesc[0:1, c0:c1],
                            start=False,
                            stop=True,
                        )
                        nc.scalar.copy(sb_K[:, ft, c0:c1], ps)
                # V tok-major over compacted keys, [tok, head, dk]
                for nh in range(2):
                    for tt in range(STC):
                        ps = ps1v.tile([128, 384], f32, tag="vp")
                        for kt in range(FT):
                            nc.tensor.matmul(
                                ps,
                                sb_xkvT[:, kt, tt * 128 : (tt + 1) * 128],
                                w_v[:, kt, nh * 384 : (nh + 1) * 384],
                                start=(kt == 0),
                                stop=False,
                            )
                        nc.tensor.matmul(
                            ps,
                            sb_onesc[0:1, tt * 128 : (tt + 1) * 128],
                            sb_bvrow[0:1, nh * 384 : (nh + 1) * 384],
                            start=False,
                            stop=True,
                        )
                        nc.vector.tensor_copy(
                            sb_V[:, tt, nh * 6 : (nh + 1) * 6, 0:DK],
                            ps[:].rearrange("p (h d) -> p h d", d=DK),
                        )

            # prefetched O-proj / FFN1 weights: DMAs are issued after the
            # phase-1 loads (below) and so overlap attention instead of
            # stalling behind it
            for kt in range(FT):
                nc.sync.dma_start(
                    out=w_o[:, kt, :], in_=d_wo[kt * 128 : (kt + 1) * 128, :]
                )
            for kt in range(FT):
                nc.sync.dma_start(
                    out=w1_t[kt], in_=d_w1[kt * 128 : (kt + 1) * 128, :]
                )

            if MAX_PHASE >= 2:
                # ============ Phase 2: attention ============
                with (
                    tc.tile_pool(name="expp", bufs=20) as expp,
                    tc.tile_pool(name="attsm", bufs=2) as attsm,
                    tc.tile_pool(name="ps_sc", bufs=2, space="PSUM") as ps_sc,
                    tc.tile_pool(name="ps_z", bufs=1, space="PSUM") as ps_z,
                    tc.tile_pool(name="ps_rb", bufs=2, space="PSUM") as ps_rb,
                ):
                    for hp in range(HEADS // 2):
                        ht = hp
                        # interleave the two heads of a pair kt-by-kt: their
                        # K=64 matmuls sit in disjoint PE row groups (0-63 /
                        # 64-127) so the hardware overlaps adjacent pairs.
                        # Both halves land in one 2-bank PSUM tile so a single
                        # [128,1024] exp covers them (halves ACT dispatch+sem
                        # overhead).
                        ets = ([], [])
                        for kt2 in range(STC):
                            ps = ps_sc.tile([128, 2 * BLK], f32, tag="sc")
                            for half in (0, 1):
                                ho = half * 64
                                nc.tensor.matmul(
                                    ps[:, half * BLK : (half + 1) * BLK],
                                    sb_K[ho : ho + 64, ht, kt2 * 128 : (kt2 + 1) * 128],
                                    sb_Q[ho : ho + 64, ht, :],
                                    start=True,
                                    stop=True,
                                )
                            et = expp.tile([128, 2 * BLK], bf16, tag="exp")
                            nc.scalar.activation(et, ps, AF.Exp, scale=ISCALE)
                            for half in (0, 1):
                                ets[half].append(
                                    et[:, half * BLK : (half + 1) * BLK]
                                )
                        zps = []
                        for half in (0, 1):
                            h = 2 * hp + half
                            zp = ps_z.tile([DK + 1, BLK], f32, tag=f"z{half}")
                            # seed with the masked-keys correction row
                            nc.tensor.matmul(
                                zp,
                                sb_crow[0:1, h * (DK + 1) : (h + 1) * (DK + 1)],
                                ones512f[:],
                                start=True,
                                stop=False,
                            )
                            for kt2 in range(STC):
                                nc.tensor.matmul(
                                    zp,
                                    sb_V[:, kt2, h, :],
                                    ets[half][kt2],
                                    start=False,
                                    stop=(kt2 == STC - 1),
                                )
                            zps.append(zp)
                        for half in (0, 1):
                            ho = half * 64
                            # denominators are huge sums (>= 1): the ~18-bit
                            # fast reciprocal is ~5x cheaper and plenty exact.
                            # (bitwise-trick op: input must be in SBUF, not PSUM)
                            zrow = attsm.tile([1, BLK], f32, tag="zrow")
                            nc.vector.tensor_copy(zrow, zps[half][DK : DK + 1, :])
                            rsum = attsm.tile([1, BLK], f32, tag="rsum")
                            nc.vector.reciprocal_approx_fast(rsum, zrow)
                            rbp = ps_rb.tile([64, BLK], f32, tag="rb")
                            nc.tensor.matmul(
                                rbp, ones64[:], rsum, start=True, stop=True
                            )
                            rb = attsm.tile([64, BLK], f32, tag="rbs")
                            nc.vector.tensor_copy(rb, rbp)
                            nc.vector.tensor_mul(
                                sb_zT[ho : ho + 64, ht, :], zps[half][0:DK, :], rb
                            )

            if MAX_PHASE >= 3:
                # ============ Phase 3: O proj + LN1 (+residual) ============
                def layer_norm_to(out_ap, x_ap, g_bc_t, resid_ap, pool):
                    s = pool.tile([128, 1], f32, tag="ln_s")
                    nc.vector.tensor_reduce(s, x_ap, axis=AX.X, op=ALU.add)
                    junk = pool.tile([128, DIM], f32, tag="ln_j")
                    ssq = pool.tile([128, 1], f32, tag="ln_q")
                    # (tensor_tensor_reduce crashes the device on this runtime;
                    # scalar_tensor_tensor with accum_out works)
                    nc.vector.scalar_tensor_tensor(
                        out=junk, in0=x_ap, scalar=1.0, in1=x_ap,
                        op0=ALU.mult, op1=ALU.mult, accum_out=ssq,
                    )
                    negmean = pool.tile([128, 1], f32, tag="ln_m")
                    nc.scalar.mul(negmean, s, -1.0 / DIM)
                    # var = E[x^2] - mean^2
                    m2 = pool.tile([128, 1], f32, tag="ln_m2")
                    nc.vector.tensor_mul(m2, negmean, negmean)
                    var = pool.tile([128, 1], f32, tag="ln_v")
                    nc.vector.scalar_tensor_tensor(
                        out=var, in0=ssq, scalar=1.0 / DIM, in1=m2,
                        op0=ALU.mult, op1=ALU.subtract,
                    )
                    sd = pool.tile([128, 1], f32, tag="ln_sd")
                    nc.scalar.activation(sd, var, AF.Sqrt, bias=eps_t[:])
                    rstd = pool.tile([128, 1], f32, tag="ln_r")
                    nc.vector.reciprocal(rstd, sd)
                    nmr = pool.tile([128, 1], f32, tag="ln_nm")
                    nc.vector.tensor_mul(nmr, negmean, rstd)
                    # x*rstd on ACT; then (x*rstd - mu*rstd) * g fused on DVE
                    nrm = pool.tile([128, DIM], f32, tag="ln_t")
                    nc.scalar.mul(nrm, x_ap, rstd[:])
                    tg = pool.tile([128, DIM], f32, tag="ln_tg")
                    nc.vector.scalar_tensor_tensor(
                        out=tg, in0=nrm, scalar=nmr[:], in1=g_bc_t,
                        op0=ALU.add, op1=ALU.mult,
                    )
                    nc.vector.tensor_add(out_ap, tg, resid_ap)

                with (
                    tc.tile_pool(name="ln1p", bufs=2) as ln1p,
                    tc.tile_pool(name="ps_o", bufs=4, space="PSUM") as ps_o,
                ):
                    for tt in range(TT):
                        l1pre = ln1p.tile([128, DIM], f32, tag="l1pre")
                        for nh in range(2):
                            ps = ps_o.tile([128, 384], f32, tag="op")
                            for kt in range(FT):
                                nc.tensor.matmul(
                                    ps,
                                    sb_zT[:, kt, tt * 128 : (tt + 1) * 128],
                                    w_o[:, kt, nh * 384 : (nh + 1) * 384],
                                    start=(kt == 0),
                                    stop=(kt == FT - 1),
                                )
                            nc.vector.scalar_tensor_tensor(
                                out=l1pre[:, nh * 384 : (nh + 1) * 384],
                                in0=ps,
                                scalar=1.0,
                                in1=bo_bc[:, nh * 384 : (nh + 1) * 384],
                                op0=ALU.mult,
                                op1=ALU.add,
                            )
                        xb1 = ln1p.tile([128, DIM], f32, tag="xb1")
                        nc.vector.tensor_add(xb1, sb_xblk[:, tt, :], bb1_bc)
                        layer_norm_to(sb_l1[:, tt, :], l1pre[:], g1_bc, xb1, ln1p)

            attn_res_cm.__exit__(None, None, None)
            hT_cm = tc.tile_pool(name="hTp", bufs=1)
            hTp = hT_cm.__enter__()
            sb_hT = hTp.tile([128, HT, BLK], bf16)  # relu(ffn1)^T, hid-major

            # prefetch FFN2 weights as soon as attention space frees
            w2pre_cm = tc.tile_pool(name="w2pre", bufs=1)
            w2pre = w2pre_cm.__enter__()
            w2_t = []
            for kt in range(HT):
                w2_kt = w2pre.tile([128, DIM], bf16, tag=f"w2_{kt}", name=f"w2_{kt}")
                nc.sync.dma_start(out=w2_kt, in_=d_w2[kt * 128 : (kt + 1) * 128, :])
                w2_t.append(w2_kt)


            if MAX_PHASE >= 4:
                # ============ Phase 4: transpose l1, FFN1 ============
                with (
                    tc.tile_pool(name="l1t_p", bufs=1) as l1t_p,
                    tc.tile_pool(name="ps_t", bufs=2, space="PSUM") as ps_t,
                    tc.tile_pool(name="ps_f1", bufs=4, space="PSUM") as ps_f1,
                ):
                    sb_l1T = l1t_p.tile([128, FT, BLK], bf16)
                    for ft in range(FT):
                        for tt in range(TT):
                            pst = ps_t.tile([128, 128], f32, tag="tp")
                            nc.tensor.transpose(
                                pst, sb_l1[:, tt, ft * 128 : (ft + 1) * 128], ident[:]
                            )
                            nc.scalar.copy(
                                sb_l1T[:, ft, tt * 128 : (tt + 1) * 128], pst
                            )
                    for ht2 in range(HT):
                        ps = ps_f1.tile([128, BLK], f32, tag="f1")
                        for kt in range(FT):
                            nc.tensor.matmul(
                                ps,
                                w1_t[kt][:, ht2 * 128 : (ht2 + 1) * 128],
                                sb_l1T[:, kt, :],
                                start=(kt == 0),
                                stop=(kt == FT - 1),
                            )
                        # relu(x + b1) on DVE: (x add b1) max 0
                        nc.vector.tensor_scalar(
                            sb_hT[:, ht2, :], ps, sb_b1[:, ht2 : ht2 + 1], 0.0,
                            op0=ALU.add, op1=ALU.max,
                        )

            if MAX_PHASE >= 5:
                # ============ Phase 5: FFN2 + LN2 + out ============
                with (
                    tc.tile_pool(name="ln2p", bufs=2) as ln2p,
                    tc.tile_pool(name="outp", bufs=3) as outp,
                    tc.tile_pool(name="ps_f2", bufs=4, space="PSUM") as ps_f2,
                ):
                    out_r = d_out[:].rearrange("(t p) d -> p t d", p=128)
                    for tt in range(TT):
                        f2pre = ln2p.tile([128, DIM], f32, tag="f2pre")
                        for nh in range(2):
                            ps = ps_f2.tile([128, 384], f32, tag="f2")
                            for kt in range(HT):
                                nc.tensor.matmul(
                                    ps,
                                    sb_hT[:, kt, tt * 128 : (tt + 1) * 128],
                                    w2_t[kt][:, nh * 384 : (nh + 1) * 384],
                                    start=(kt == 0),
                                    stop=(kt == HT - 1),
                                )
                            nc.vector.scalar_tensor_tensor(
                                out=f2pre[:, nh * 384 : (nh + 1) * 384],
                                in0=ps,
                                scalar=1.0,
                                in1=b2_bc[:, nh * 384 : (nh + 1) * 384],
                                op0=ALU.mult,
                                op1=ALU.add,
                            )
                        l1b = ln2p.tile([128, DIM], f32, tag="l1b")
                        nc.vector.tensor_add(l1b, sb_l1[:, tt, :], bb2_bc)
                        o_sb = outp.tile([128, DIM], f32, tag="osb")
                        layer_norm_to(o_sb[:], f2pre[:], g2_bc, l1b, ln2p)
                        nc.sync.dma_start(out=out_r[:, tt, :], in_=o_sb)

            w2pre_cm.__exit__(None, None, None)
            hT_cm.__exit__(None, None, None)
            wpre_cm.__exit__(None, None, None)

    return nc


def _get_nc(finalized=True):
    if "nc" not in _CACHE:
        _CACHE["nc"] = _build_program()
    nc = _CACHE["nc"]
    if finalized and not nc.is_finalized():
        nc.finalize()
    return nc


def make_in_maps(inputs: dict) -> list:
    x = np.asarray(inputs["x_n"], np.float32).reshape(B, S, DIM)
    mask = np.asarray(inputs["mask"]).reshape(B, S)
    w = {
        k: np.ascontiguousarray(np.asarray(inputs[k], np.float32).astype(BF16))
        for k in ("wq", "wk", "wv", "wo", "w1", "w2")
    }
    vecs = {
        "bo": inputs["bo"], "b1": inputs["b1"], "b2": inputs["b2"],
        "g1": inputs["ln1_g"], "bb1": inputs["ln1_b"],
        "g2": inputs["ln2_g"], "bb2": inputs["ln2_b"],
    }
    vecs = {k: np.ascontiguousarray(np.asarray(v, np.float32)) for k, v in vecs.items()}
    brows = {
        "bkrow": np.asarray(inputs["bk"], np.float32).astype(BF16),
        "bvrow": np.asarray(inputs["bv"], np.float32).astype(BF16),
        "bqrow": np.asarray(inputs["bq"], np.float32).astype(BF16),
    }

    # per-batch compaction + masked-keys correction
    per_batch = []
    for b in range(B):
        mb = mask[b] != 0
        idx = np.nonzero(mb)[0]
        n_u = len(idx)
        if n_u > KC:
            raise RuntimeError(
                f"unmasked key count {n_u} exceeds compiled capacity {KC}"
            )
        xkv = np.zeros((KC, DIM), np.float32)
        xkv[:n_u] = x[b][idx]
        xkvT = np.ascontiguousarray(xkv.T.astype(BF16))
        onesc = np.zeros(KC, np.float32)
        onesc[:n_u] = 1.0
        msum = x[b][~mb].astype(np.float64).sum(axis=0)
        mcount = float((~mb).sum())
        wv64 = np.asarray(inputs["wv"], np.float64)
        bv64 = np.asarray(inputs["bv"], np.float64)
        cvec = (msum @ wv64 + mcount * bv64).astype(np.float32)  # [DIM]
        crow = np.zeros(HEADS * (DK + 1), np.float32)
        ch = cvec.reshape(HEADS, DK)
        for h in range(HEADS):
            crow[h * (DK + 1) : h * (DK + 1) + DK] = ch[h]
            crow[h * (DK + 1) + DK] = mcount
        per_batch.append(
            {"xkvT": xkvT, "onesc": onesc.astype(BF16), "crow": crow}
        )

    in_maps = []
    for c in range(N_CORES):
        b, blk = c // NBLK, c % NBLK
        xb = x[b]
        xblk = np.ascontiguousarray(xb[blk * BLK : (blk + 1) * BLK])
        xTb = np.ascontiguousarray(xblk.T.astype(BF16))
        m = {"xTb": xTb, "xb": xblk}
        m.update(per_batch[b])
        m.update(w)
        m.update(vecs)
        m.update(brows)
        in_maps.append(m)
    return in_maps


def assemble(per_core_out: list) -> np.ndarray:
    blocks = [np.asarray(o, np.float32) for o in per_core_out]
    full = np.concatenate(blocks, axis=0).reshape(B, S, DIM)
    return full


def kernel(**inputs) -> np.ndarray:
    from concourse.bass_utils import run_bass_kernel_spmd

    nc = _get_nc()
    in_maps = make_in_maps(inputs)
    res = run_bass_kernel_spmd(nc, in_maps, list(range(N_CORES)))
    return assemble([r["out"] for r in res.results])


# revision 32
# speedup vs baseline: 1.8738x; 1.0197x over previous
"""Trainium2 Bass kernel for a dense transformer encoder layer (v2).

Model (faithful to the oracle):
  q,k,v = x@wq+bq, x@wk+bk, x@wv+bv          (12 heads, dk=64, DIM=768)
  scores = q@k^T / sqrt(768)  (note: sqrt(dim_model), not sqrt(dk))
  scores[mask==0] = 1e-11  (NOT -inf; masked keys still contribute ~1/Z)
  attn = softmax(scores); z = attn@v; o = z@wo+bo
  l1 = x + LN(o);  ffn = relu(l1@w1+b1)@w2+b2;  out = l1 + LN(ffn)

Sharding: 4096 tokens (B=2,S=2048) split 8 ways -> 512 tokens/core.
Cores 0-3 own batch 0, cores 4-7 batch 1. No collectives: each core
computes K/V itself — but only for the UNMASKED keys of its batch.

Mask compaction: masked keys (~half) all get score 1e-11, i.e. exp==1
(fp32), for every query/head. Their attention contribution is a
query-independent constant: C_h = sum_masked v_k (numerator) and
m = #masked (denominator). The host compacts unmasked tokens into a
padded [KC] buffer (KC=1280 >> max plausible count), computes the tiny
C correction in numpy, and the device runs attention only over the
compacted keys, seeding the attn@v PSUM accumulation with C via a K=1
matmul. Padded key slots have K=V=0 exactly (biases are added inside
the matmuls via a 0/1 validity row), so exp(score)=1 * V=0 contributes
nothing; the validity row also zeroes their denominator entry.

Softmax: scores are built k-major (scoresT [kpos, q]); all compacted
keys are unmasked so exp(ISCALE * s) uses a constant scale. The
denominator comes from a ones column in V (1 for real keys, 0 for
pads, via one replicated DMA); normalization happens after attn@v via
a rank-1 matmul broadcast of the fast reciprocal (input staged to SBUF
— the bit-trick op misreads PSUM).
"""

import math
import os
import sys

import numpy as np

for _p in ("/opt/trn_rl_repo", os.path.expanduser("~/.axon_site/_ro/trn_rl_repo")):
    if os.path.isdir(_p) and _p not in sys.path:
        sys.path.insert(0, _p)

import ml_dtypes  # noqa: E402

BF16 = ml_dtypes.bfloat16

DIM = 768
HEADS = 12
DK = 64
HID = 4 * DIM  # 3072
B, S = 2, 2048
N_CORES = 8
BLK = 512            # tokens per core
NBLK = S // BLK      # 4 blocks per batch
KC = 1152            # compacted-key capacity (9 tiles; ~5.7 sigma above E[n_u])
EPS = 1e-5
ISCALE = 1.0 / math.sqrt(DIM)

FT = DIM // 128   # 6 feature tiles
TT = BLK // 128   # 4 token tiles per core block
STC = KC // 128   # 10 compacted key tiles
HT = HID // 128   # 24 hidden tiles

_CACHE: dict = {}
MAX_PHASE = int(os.environ.get("BASS_KERNEL_PHASES", "5"))


def _build_program():
    import concourse.bass as bass
    import concourse.mybir as mybir
    import concourse.tile as tile
    from concourse import bacc
    from concourse.masks import make_identity

    f32 = mybir.dt.float32
    bf16 = mybir.dt.bfloat16
    AF = mybir.ActivationFunctionType
    ALU = mybir.AluOpType
    AX = mybir.AxisListType

    nc = bacc.Bacc()

    # ---- per-core DRAM I/O ----
    d_xTb = nc.dram_tensor("xTb", [DIM, BLK], bf16, kind="ExternalInput")
    d_xb = nc.dram_tensor("xb", [BLK, DIM], f32, kind="ExternalInput")
    d_xkvT = nc.dram_tensor("xkvT", [DIM, KC], bf16, kind="ExternalInput")
    d_onesc = nc.dram_tensor("onesc", [KC], bf16, kind="ExternalInput")
    d_wq = nc.dram_tensor("wq", [DIM, DIM], bf16, kind="ExternalInput")
    d_wk = nc.dram_tensor("wk", [DIM, DIM], bf16, kind="ExternalInput")
    d_wv = nc.dram_tensor("wv", [DIM, DIM], bf16, kind="ExternalInput")
    d_wo = nc.dram_tensor("wo", [DIM, DIM], bf16, kind="ExternalInput")
    d_w1 = nc.dram_tensor("w1", [DIM, HID], bf16, kind="ExternalInput")
    d_w2 = nc.dram_tensor("w2", [HID, DIM], bf16, kind="ExternalInput")
    d_bkrow = nc.dram_tensor("bkrow", [DIM], bf16, kind="ExternalInput")
    d_bvrow = nc.dram_tensor("bvrow", [DIM], bf16, kind="ExternalInput")
    d_bqrow = nc.dram_tensor("bqrow", [DIM], bf16, kind="ExternalInput")
    d_crow = nc.dram_tensor("crow", [HEADS * (DK + 1)], f32, kind="ExternalInput")
    d_bo = nc.dram_tensor("bo", [DIM], f32, kind="ExternalInput")
    d_b1 = nc.dram_tensor("b1", [HID], f32, kind="ExternalInput")
    d_b2 = nc.dram_tensor("b2", [DIM], f32, kind="ExternalInput")
    d_g1 = nc.dram_tensor("g1", [DIM], f32, kind="ExternalInput")
    d_bb1 = nc.dram_tensor("bb1", [DIM], f32, kind="ExternalInput")
    d_g2 = nc.dram_tensor("g2", [DIM], f32, kind="ExternalInput")
    d_bb2 = nc.dram_tensor("bb2", [DIM], f32, kind="ExternalInput")
    d_out = nc.dram_tensor("out", [BLK, DIM], f32, kind="ExternalOutput")

    KCH = [(0, 512), (512, 1024), (1024, KC)]  # K-proj N chunks

    def bcast_ap(handle, n=128):
        ap = handle[:]
        return bass.AP(tensor=ap.tensor, offset=ap.offset, ap=[[0, n]] + list(ap.ap))

    with tile.TileContext(nc) as tc:
        with (
            tc.tile_pool(name="const", bufs=1) as const,
            tc.tile_pool(name="bigres", bufs=1) as big,
        ):
            # ---------- constants ----------
            # (row/bias constants ride the gpsimd queue or late sync slots;
            # the sync queue head is reserved for phase-1-critical loads)
            sb_b1 = const.tile([128, HT], f32)
            bo_bc = const.tile([128, DIM], bf16)
            nc.gpsimd.dma_start(out=bo_bc, in_=bcast_ap(d_bo))
            b2_bc = const.tile([128, DIM], bf16)
            nc.gpsimd.dma_start(out=b2_bc, in_=bcast_ap(d_b2))
            g1_bc = const.tile([128, DIM], bf16)
            nc.gpsimd.dma_start(out=g1_bc, in_=bcast_ap(d_g1))
            bb1_bc = const.tile([128, DIM], bf16)
            nc.gpsimd.dma_start(out=bb1_bc, in_=bcast_ap(d_bb1))
            g2_bc = const.tile([128, DIM], bf16)
            nc.gpsimd.dma_start(out=g2_bc, in_=bcast_ap(d_g2))
            bb2_bc = const.tile([128, DIM], bf16)
            nc.gpsimd.dma_start(out=bb2_bc, in_=bcast_ap(d_bb2))
            ident = const.tile([128, 128], f32)
            make_identity(nc, ident[:])
            ones64 = const.tile([1, 64], f32)
            nc.vector.memset(ones64, 1.0)
            ones512f = const.tile([1, BLK], f32)
            nc.vector.memset(ones512f, 1.0)
            ones512b = const.tile([1, BLK], bf16)
            nc.vector.memset(ones512b, 1.0)
            eps_t = const.tile([128, 1], f32)
            nc.vector.memset(eps_t, EPS)
            sb_bkrow = const.tile([1, DIM], bf16)
            sb_bvrow = const.tile([1, DIM], bf16)
            sb_bqrow = const.tile([1, DIM], bf16)
            sb_crow = const.tile([1, HEADS * (DK + 1)], f32)
            sb_onesc = const.tile([1, KC], bf16)
            sb_onescT = const.tile([128, STC], bf16)

            # ---------- persistent activations ----------
            sb_xblk = big.tile([128, TT, DIM], f32)  # residual x
            sb_l1 = big.tile([128, TT, DIM], f32)

            # weight-prefetch pool: opened before attn_res so pool pops
            # stay LIFO (attn_res dies first); DMAs are issued after phase 1
            wpre_cm = tc.tile_pool(name="wpre", bufs=1)
            wpre = wpre_cm.__enter__()
            w_o = wpre.tile([128, FT, DIM], bf16)
            w1_t = []
            for kt in range(FT):
                w1_kt = wpre.tile([128, HID], bf16, tag=f"w1_{kt}", name=f"w1_{kt}")
                w1_t.append(w1_kt)


            # attention-scoped residents (freed before the FFN phases)
            attn_res_cm = tc.tile_pool(name="attn_res", bufs=1)
            attn_res = attn_res_cm.__enter__()
            sb_K = attn_res.tile([128, FT, KC], bf16)  # K^T, feat-major
            sb_Q = attn_res.tile([128, FT, BLK], bf16)  # Q^T, feat-major
            sb_V = attn_res.tile([128, STC, HEADS, DK + 1], bf16)  # V + ones col
            sb_zT = attn_res.tile([128, FT, BLK], bf16)  # z^T normalized


            # ===== Phase 1+2: QKV projections fused with attention =====
            # PE stream: Q, V, then per head-pair {K(ft), attnV(prev), scores}
            # so exp (ACT) and the softmax tail (DVE) overlap the next pair's
            # projection/score matmuls instead of serializing phase-by-phase.
            with (
                tc.tile_pool(name="xw", bufs=1) as xw,
                tc.tile_pool(name="expp", bufs=20) as expp,
                tc.tile_pool(name="attsm", bufs=1) as attsm,
                tc.tile_pool(name="ps1", bufs=2, space="PSUM") as ps1,
                tc.tile_pool(name="ps_sc", bufs=3, space="PSUM") as ps_sc,
                tc.tile_pool(name="ps_z", bufs=1, space="PSUM") as ps_z,
                tc.tile_pool(name="ps_rb", bufs=1, space="PSUM") as ps_rb,
            ):
                # sync-queue order = need order: Q inputs, V inputs, K inputs,
                # then phase-3+ constants, then prefetched phase-3/4 weights.
                sb_xTb = xw.tile([128, FT, BLK], bf16)
                nc.sync.dma_start(
                    out=sb_xTb, in_=d_xTb[:].rearrange("(t p) n -> p t n", p=128)
                )
                nc.sync.dma_start(out=sb_bqrow, in_=bcast_ap(d_bqrow, 1))
                w_q = xw.tile([128, FT, DIM], bf16)
                for kt in range(FT):
                    nc.sync.dma_start(
                        out=w_q[:, kt, :], in_=d_wq[kt * 128 : (kt + 1) * 128, :]
                    )
                sb_xkvT = xw.tile([128, FT, KC], bf16)
                w_v = xw.tile([128, FT, DIM], bf16)
                for kt in range(FT):
                    nc.sync.dma_start(
                        out=sb_xkvT[:, kt, :],
                        in_=d_xkvT[kt * 128 : (kt + 1) * 128, :],
                    )
                    nc.sync.dma_start(
                        out=w_v[:, kt, :], in_=d_wv[kt * 128 : (kt + 1) * 128, :]
                    )
                nc.sync.dma_start(out=sb_onesc, in_=bcast_ap(d_onesc, 1))
                nc.sync.dma_start(out=sb_bvrow, in_=bcast_ap(d_bvrow, 1))
                nc.sync.dma_start(
                    out=sb_onescT, in_=d_onesc[:].rearrange("(t p) -> p t", p=128)
                )
                # ones column of V: 12 cheap strided copies from the dense
                # tok-major validity tile (a direct strided DMA generates
                # 2-byte-packet descriptor spam and a ~26us drain)
                for h in range(HEADS):
                    nc.vector.tensor_copy(
                        sb_V[:, :, h, DK : DK + 1],
                        sb_onescT[:].rearrange("p (t o) -> p t o", o=1),
                    )
                w_k = xw.tile([128, FT, DIM], bf16)
                for kt in range(FT):
                    nc.sync.dma_start(
                        out=w_k[:, kt, :], in_=d_wk[kt * 128 : (kt + 1) * 128, :]
                    )
                nc.sync.dma_start(out=sb_bkrow, in_=bcast_ap(d_bkrow, 1))
                nc.sync.dma_start(out=sb_crow, in_=bcast_ap(d_crow, 1))
                nc.sync.dma_start(
                    out=sb_xblk, in_=d_xb[:].rearrange("(t p) d -> p t d", p=128)
                )
                nc.sync.dma_start(
                    out=sb_b1, in_=d_b1[:].rearrange("(t p) -> p t", p=128)
                )
                # prefetched O-proj / FFN1 weights (into wpre, which has no
                # SBUF overlap with attention pools): overlap attention
                for kt in range(FT):
                    nc.sync.dma_start(
                        out=w_o[:, kt, :], in_=d_wo[kt * 128 : (kt + 1) * 128, :]
                    )
                for kt in range(FT):
                    nc.sync.dma_start(
                        out=w1_t[kt], in_=d_w1[kt * 128 : (kt + 1) * 128, :]
                    )

                # Q^T feat-major for the core's block (inputs land first)
                for ft in range(FT):
                    ps = ps1.tile([128, BLK], f32, tag="p")
                    for kt in range(FT):
                        nc.tensor.matmul(
                            ps,
                            w_q[:, kt, ft * 128 : (ft + 1) * 128],
                            sb_xTb[:, kt, :],
                            start=(kt == 0),
                            stop=False,
                        )
                    nc.tensor.matmul(
                        ps,
                        sb_bqrow[0:1, ft * 128 : (ft + 1) * 128],
                        ones512b[:],
                        start=False,
                        stop=True,
                    )
                    nc.scalar.copy(sb_Q[:, ft, :], ps)
                # V tok-major over compacted keys, [tok, head, dk]
                for nh in range(2):
                    for tt in range(STC):
                        ps = ps1.tile([128, 384], f32, tag="p")
                        for kt in range(FT):
                            nc.tensor.matmul(
                                ps,
                                sb_xkvT[:, kt, tt * 128 : (tt + 1) * 128],
                                w_v[:, kt, nh * 384 : (nh + 1) * 384],
                                start=(kt == 0),
                                stop=False,
                            )
                        nc.tensor.matmul(
                            ps,
                            sb_onesc[0:1, tt * 128 : (tt + 1) * 128],
                            sb_bvrow[0:1, nh * 384 : (nh + 1) * 384],
                            start=False,
                            stop=True,
                        )
                        nc.vector.tensor_copy(
                            sb_V[:, tt, nh * 6 : (nh + 1) * 6, 0:DK],
                            ps[:].rearrange("p (h d) -> p h d", d=DK),
                        )

                def k_proj(ft):
                    for c0, c1 in KCH:
                        ps = ps1.tile([128, c1 - c0], f32, tag="p", name="ps_k")
                        for kt in range(FT):
                            nc.tensor.matmul(
                                ps,
                                w_k[:, kt, ft * 128 : (ft + 1) * 128],
                                sb_xkvT[:, kt, c0:c1],
                                start=(kt == 0),
                                stop=False,
                            )
                        nc.tensor.matmul(
                            ps,
                            sb_bkrow[0:1, ft * 128 : (ft + 1) * 128],
                            sb_onesc[0:1, c0:c1],
                            start=False,
                            stop=True,
                        )
                        nc.scalar.copy(sb_K[:, ft, c0:c1], ps)

                def scores(hp):
                    ets = ([], [])
                    for kt2 in range(STC):
                        for half in (0, 1):
                            ho = half * 64
                            ps = ps_sc.tile([128, BLK], f32, tag="sc", name="ps_s")
                            nc.tensor.matmul(
                                ps,
                                sb_K[ho : ho + 64, hp, kt2 * 128 : (kt2 + 1) * 128],
                                sb_Q[ho : ho + 64, hp, :],
                                start=True,
                                stop=True,
                            )
                            et = expp.tile([128, BLK], bf16, tag="exp", name="et")
                            nc.scalar.activation(et, ps, AF.Exp, scale=ISCALE)
                            ets[half].append(et)
                    return ets

                def attn_v(hp, ets):
                    zps = []
                    for half in (0, 1):
                        h = 2 * hp + half
                        zp = ps_z.tile([DK + 1, BLK], f32, tag=f"z{half}",
                                       name="ps_z")
                        # seed with the masked-keys correction row
                        nc.tensor.matmul(
                            zp,
                            sb_crow[0:1, h * (DK + 1) : (h + 1) * (DK + 1)],
                            ones512f[:],
                            start=True,
                            stop=False,
                        )
                        for kt2 in range(STC):
                            nc.tensor.matmul(
                                zp,
                                sb_V[:, kt2, h, :],
                                ets[half][kt2],
                                start=False,
                                stop=(kt2 == STC - 1),
                            )
                        zps.append(zp)
                    rsums = []
                    for half in (0, 1):
                        # denominators are huge sums (>= 1): the ~18-bit fast
                        # reciprocal is ~5x cheaper and plenty exact.
                        # (bitwise-trick op: input must be in SBUF, not PSUM)
                        zrow = attsm.tile([1, BLK], f32, tag="zrow", name="zrow")
                        nc.vector.tensor_copy(zrow, zps[half][DK : DK + 1, :])
                        rsum = attsm.tile([1, BLK], f32, tag="rsum", name="rsum")
                        nc.vector.reciprocal_approx_fast(rsum, zrow)
                        rsums.append(rsum)
                    for half in (0, 1):
                        ho = half * 64
                        rbp = ps_rb.tile([64, BLK], f32, tag="rb", name="ps_rb")
                        nc.tensor.matmul(
                            rbp, ones64[:], rsums[half], start=True, stop=True
                        )
                        rb = attsm.tile([64, BLK], f32, tag="rbs", name="rb")
                        nc.vector.tensor_copy(rb, rbp)
                        nc.vector.tensor_mul(
                            sb_zT[ho : ho + 64, hp, :], zps[half][0:DK, :], rb
                        )

                k_proj(0)
                ets_prev = scores(0)
                for hp in range(1, HEADS // 2):
                    k_proj(hp)
                    attn_v(hp - 1, ets_prev)
                    ets_prev = scores(hp)
                attn_v(HEADS // 2 - 1, ets_prev)

            if MAX_PHASE >= 3:
                # ============ Phase 3: O proj + LN1 (+residual) ============
                def layer_norm_to(out_ap, x_ap, g_bc_t, resid_ap, pool):
                    s = pool.tile([128, 1], f32, tag="ln_s")
                    nc.vector.tensor_reduce(s, x_ap, axis=AX.X, op=ALU.add)
                    junk = pool.tile([128, DIM], f32, tag="ln_j")
                    ssq = pool.tile([128, 1], f32, tag="ln_q")
                    # (tensor_tensor_reduce crashes the device on this runtime;
                    # scalar_tensor_tensor with accum_out works)
                    nc.vector.scalar_tensor_tensor(
                        out=junk, in0=x_ap, scalar=1.0, in1=x_ap,
                        op0=ALU.mult, op1=ALU.mult, accum_out=ssq,
                    )
                    negmean = pool.tile([128, 1], f32, tag="ln_m")
                    nc.scalar.mul(negmean, s, -1.0 / DIM)
                    # var = E[x^2] - mean^2
                    m2 = pool.tile([128, 1], f32, tag="ln_m2")
                    nc.vector.tensor_mul(m2, negmean, negmean)
                    var = pool.tile([128, 1], f32, tag="ln_v")
                    nc.vector.scalar_tensor_tensor(
                        out=var, in0=ssq, scalar=1.0 / DIM, in1=m2,
                        op0=ALU.mult, op1=ALU.subtract,
                    )
                    sd = pool.tile([128, 1], f32, tag="ln_sd")
                    nc.scalar.activation(sd, var, AF.Sqrt, bias=eps_t[:])
                    rstd = pool.tile([128, 1], f32, tag="ln_r")
                    nc.vector.reciprocal(rstd, sd)
                    nmr = pool.tile([128, 1], f32, tag="ln_nm")
                    nc.vector.tensor_mul(nmr, negmean, rstd)
                    # x*rstd on ACT; then (x*rstd - mu*rstd) * g fused on DVE
                    nrm = pool.tile([128, DIM], f32, tag="ln_t")
                    nc.scalar.mul(nrm, x_ap, rstd[:])
                    tg = pool.tile([128, DIM], f32, tag="ln_tg")
                    nc.vector.scalar_tensor_tensor(
                        out=tg, in0=nrm, scalar=nmr[:], in1=g_bc_t,
                        op0=ALU.add, op1=ALU.mult,
                    )
                    nc.vector.tensor_add(out_ap, tg, resid_ap)

                with (
                    tc.tile_pool(name="ln1p", bufs=2) as ln1p,
                    tc.tile_pool(name="ps_o", bufs=4, space="PSUM") as ps_o,
                ):
                    for tt in range(TT):
                        l1pre = ln1p.tile([128, DIM], f32, tag="l1pre")
                        for nh in range(2):
                            ps = ps_o.tile([128, 384], f32, tag="op")
                            for kt in range(FT):
                                nc.tensor.matmul(
                                    ps,
                                    sb_zT[:, kt, tt * 128 : (tt + 1) * 128],
                                    w_o[:, kt, nh * 384 : (nh + 1) * 384],
                                    start=(kt == 0),
                                    stop=(kt == FT - 1),
                                )
                            nc.vector.scalar_tensor_tensor(
                                out=l1pre[:, nh * 384 : (nh + 1) * 384],
                                in0=ps,
                                scalar=1.0,
                                in1=bo_bc[:, nh * 384 : (nh + 1) * 384],
                                op0=ALU.mult,
                                op1=ALU.add,
                            )
                        xb1 = ln1p.tile([128, DIM], f32, tag="xb1")
                        nc.vector.tensor_add(xb1, sb_xblk[:, tt, :], bb1_bc)
                        layer_norm_to(sb_l1[:, tt, :], l1pre[:], g1_bc, xb1, ln1p)

            attn_res_cm.__exit__(None, None, None)
            hT_cm = tc.tile_pool(name="hTp", bufs=1)
            hTp = hT_cm.__enter__()
            sb_hT = hTp.tile([128, HT, BLK], bf16)  # relu(ffn1)^T, hid-major

            # prefetch FFN2 weights as soon as attention space frees
            w2pre_cm = tc.tile_pool(name="w2pre", bufs=1)
            w2pre = w2pre_cm.__enter__()
            w2_t = []
            for kt in range(HT):
                w2_kt = w2pre.tile([128, DIM], bf16, tag=f"w2_{kt}", name=f"w2_{kt}")
                nc.sync.dma_start(out=w2_kt, in_=d_w2[kt * 128 : (kt + 1) * 128, :])
                w2_t.append(w2_kt)


            if MAX_PHASE >= 4:
                # ============ Phase 4: transpose l1, FFN1 ============
                with (
                    tc.tile_pool(name="l1t_p", bufs=1) as l1t_p,
                    tc.tile_pool(name="ps_t", bufs=2, space="PSUM") as ps_t,
                    tc.tile_pool(name="ps_f1", bufs=4, space="PSUM") as ps_f1,
                ):
                    sb_l1T = l1t_p.tile([128, FT, BLK], bf16)
                    for ft in range(FT):
                        for tt in range(TT):
                            pst = ps_t.tile([128, 128], f32, tag="tp")
                            nc.tensor.transpose(
                                pst, sb_l1[:, tt, ft * 128 : (ft + 1) * 128], ident[:]
                            )
                            nc.scalar.copy(
                                sb_l1T[:, ft, tt * 128 : (tt + 1) * 128], pst
                            )
                    for ht2 in range(HT):
                        ps = ps_f1.tile([128, BLK], f32, tag="f1")
                        for kt in range(FT):
                            nc.tensor.matmul(
                                ps,
                                w1_t[kt][:, ht2 * 128 : (ht2 + 1) * 128],
                                sb_l1T[:, kt, :],
                                start=(kt == 0),
                                stop=(kt == FT - 1),
                            )
                        # relu(x + b1) on DVE: (x add b1) max 0
                        nc.vector.tensor_scalar(
                            sb_hT[:, ht2, :], ps, sb_b1[:, ht2 : ht2 + 1], 0.0,
                            op0=ALU.add, op1=ALU.max,
                        )

            if MAX_PHASE >= 5:
                # ============ Phase 5: FFN2 + LN2 + out ============
                with (
                    tc.tile_pool(name="ln2p", bufs=2) as ln2p,
                    tc.tile_pool(name="outp", bufs=3) as outp,
                    tc.tile_pool(name="ps_f2", bufs=4, space="PSUM") as ps_f2,
                ):
                    out_r = d_out[:].rearrange("(t p) d -> p t d", p=128)
                    for tt in range(TT):
                        f2pre = ln2p.tile([128, DIM], f32, tag="f2pre")
                        for nh in range(2):
                            ps = ps_f2.tile([128, 384], f32, tag="f2")
                            for kt in range(HT):
                                nc.tensor.matmul(
                                    ps,
                                    sb_hT[:, kt, tt * 128 : (tt + 1) * 128],
                                    w2_t[kt][:, nh * 384 : (nh + 1) * 384],
                                    start=(kt == 0),
                                    stop=(kt == HT - 1),
                                )
                            nc.vector.scalar_tensor_tensor(
                                out=f2pre[:, nh * 384 : (nh + 1) * 384],
                                in0=ps,
                                scalar=1.0,
                                in1=b2_bc[:, nh * 384 : (nh + 1) * 384],
                                op0=ALU.mult,
                                op1=ALU.add,
                            )
                        l1b = ln2p.tile([128, DIM], f32, tag="l1b")
                        nc.vector.tensor_add(l1b, sb_l1[:, tt, :], bb2_bc)
                        o_sb = outp.tile([128, DIM], f32, tag="osb")
                        layer_norm_to(o_sb[:], f2pre[:], g2_bc, l1b, ln2p)
                        nc.sync.dma_start(out=out_r[:, tt, :], in_=o_sb)

            w2pre_cm.__exit__(None, None, None)
            hT_cm.__exit__(None, None, None)
            wpre_cm.__exit__(None, None, None)

    return nc


def _get_nc(finalized=True):
    if "nc" not in _CACHE:
        _CACHE["nc"] = _build_program()
    nc = _CACHE["nc"]
    if finalized and not nc.is_finalized():
        nc.finalize()
    return nc


def make_in_maps(inputs: dict) -> list:
    x = np.asarray(inputs["x_n"], np.float32).reshape(B, S, DIM)
    mask = np.asarray(inputs["mask"]).reshape(B, S)
    w = {
        k: np.ascontiguousarray(np.asarray(inputs[k], np.float32).astype(BF16))
        for k in ("wq", "wk", "wv", "wo", "w1", "w2")
    }
    vecs = {
        "bo": inputs["bo"], "b1": inputs["b1"], "b2": inputs["b2"],
        "g1": inputs["ln1_g"], "bb1": inputs["ln1_b"],
        "g2": inputs["ln2_g"], "bb2": inputs["ln2_b"],
    }
    vecs = {k: np.ascontiguousarray(np.asarray(v, np.float32)) for k, v in vecs.items()}
    brows = {
        "bkrow": np.asarray(inputs["bk"], np.float32).astype(BF16),
        "bvrow": np.asarray(inputs["bv"], np.float32).astype(BF16),
        "bqrow": np.asarray(inputs["bq"], np.float32).astype(BF16),
    }

    # per-batch compaction + masked-keys correction
    per_batch = []
    for b in range(B):
        mb = mask[b] != 0
        idx = np.nonzero(mb)[0]
        n_u = len(idx)
        if n_u > KC:
            raise RuntimeError(
                f"unmasked key count {n_u} exceeds compiled capacity {KC}"
            )
        xkv = np.zeros((KC, DIM), np.float32)
        xkv[:n_u] = x[b][idx]
        xkvT = np.ascontiguousarray(xkv.T.astype(BF16))
        onesc = np.zeros(KC, np.float32)
        onesc[:n_u] = 1.0
        msum = x[b][~mb].astype(np.float64).sum(axis=0)
        mcount = float((~mb).sum())
        wv64 = np.asarray(inputs["wv"], np.float64)
        bv64 = np.asarray(inputs["bv"], np.float64)
        cvec = (msum @ wv64 + mcount * bv64).astype(np.float32)  # [DIM]
        crow = np.zeros(HEADS * (DK + 1), np.float32)
        ch = cvec.reshape(HEADS, DK)
        for h in range(HEADS):
            crow[h * (DK + 1) : h * (DK + 1) + DK] = ch[h]
            crow[h * (DK + 1) + DK] = mcount
        per_batch.append(
            {"xkvT": xkvT, "onesc": onesc.astype(BF16), "crow": crow}
        )

    in_maps = []
    for c in range(N_CORES):
        b, blk = c // NBLK, c % NBLK
        xb = x[b]
        xblk = np.ascontiguousarray(xb[blk * BLK : (blk + 1) * BLK])
        xTb = np.ascontiguousarray(xblk.T.astype(BF16))
        m = {"xTb": xTb, "xb": xblk}
        m.update(per_batch[b])
        m.update(w)
        m.update(vecs)
        m.update(brows)
        in_maps.append(m)
    return in_maps


def assemble(per_core_out: list) -> np.ndarray:
    blocks = [np.asarray(o, np.float32) for o in per_core_out]
    full = np.concatenate(blocks, axis=0).reshape(B, S, DIM)
    return full


def kernel(**inputs) -> np.ndarray:
    from concourse.bass_utils import run_bass_kernel_spmd

    nc = _get_nc()
    in_maps = make_in_maps(inputs)
    res = run_bass_kernel_spmd(nc, in_maps, list(range(N_CORES)))
    return assemble([r["out"] for r in res.results])
